# revision 30
# baseline (speedup 1.0000x reference)
"""Trainium2 Bass kernel for the CSNN (spiking CNN) problem.

Event-driven fast path: spiking networks are sparse by design, and layer 1
receives a time-constant drive c1 = conv1(x)+b1, so a sample can EVER emit
a layer-1 spike iff max_f 16*c1_f >= theta1.  When the bias-only (zero
input) trajectory is silent (host-checked from the weights), network
activity reduces to a single on-device check (build_check_program_raw):
a hand-scheduled raw-bass program (no Tile barriers) whose critical path
is ONE packed fp8e4m3 input DMA (x block-packed 14 sample groups deep,
conv1 weights AND the -tau threshold row fused into the same [128, 843]
tensor), three block-diagonal matmuls computing psum = conv1_raw(x) - tau
(7 samples per PE column; bank cols 512/454/205 balance the two scan
engines), a two-engine parallel scan
(DVE reduce_max on banks 0+2, ACT Sign+accumulate on bank 1, different
banks so the reads overlap), and ONE [126, 3] flag DMA out.  The silent
case (this problem's data: max 16*c1 = 9.8 vs theta = 20) returns the
exact all-zero output; any potentially-firing sample falls back to the
exact dense wavefront kernel below.  The tau margin rigorously absorbs
fp8 rounding of x/M1/tau (computed from the actual casts + maxabs(x)) and
fp32 accumulation-order drift, so the flag can false-fire (slow but
correct) yet never miss a real spike.  ~7.35us single-shot in the TRN2
cost model vs ~13.3us for the session-1 Tile version; Tile-loop HW
steady-state ~2.2us/iter (body_unroll=16, alternating HWDGE rings).

Dense path network (per sample, T=16 timesteps, all spatial dims 3x3):
  conv1(1->2) -> IF(20) -> conv2(2->2) -> IF(10) -> conv3(2->2) -> IF(8)
  -> conv4(2->1) -> IF(8) -> fc1(9->10) -> IF(30) -> fc2(10->2) -> IF(30)
  output = mean_t spikes6  [N, 2]

Every conv is a 3x3 SAME conv on a 3x3 image, i.e. a dense linear map on the
9*C flattened features.  The whole per-timestep network is therefore a chain
of six small matmuls plus elementwise integrate-and-fire updates.

Kernel formulation (per core, pure data parallel over the batch):
  - One block-diagonal "mega" weight matrix Wblk [85 x 77] evaluates ALL six
    layers at once in a layer-pipelined (wavefront) schedule: at step k,
    layer l processes timestep t = k - (l-1).  fp32r matmuls (full-rate fp32).
  - rhs tile [85 x 1024]: rows 0..74 = spike rows (aligned with the membrane
    rows in PSUM), rows 75..83 = the 9 input pixels (static), row 84 = ones
    (bias input).  1024 samples span two PSUM banks (2 matmuls per step).
  - Membrane potentials v live in PSUM rows 0..74 and are accumulated by the
    matmul itself (start=False).  Rows 75..76 accumulate the layer-6 spikes
    scaled by 1/T (the final output) across steps - also free via matmul.
  - Default mode sigma_clamp needs only TWO elementwise passes per step, one
    per engine:
      ScalarE:  sigma = sign(v - thr) -> rhs spike rows ({-1,+1}; weights are
                rewired for s=(sigma+1)/2, so -1 rows contribute exactly 0)
      VectorE:  v = min(v, thr) - combined with a -thr*I diagonal feedback
                block in Wblk this is an EXACT hard reset: the clamp pins v
                to exactly thr at spike time, so the next step's -thr*sigma
                feedback zeroes it.
    (Caveat: at an exact fp32 tie v == thr, sign() returns 0, encoding half a
    spike; measure-zero and irrelevant at this problem's threshold margins.)
  - Warmup bias over-accumulation (each layer receives its bias on every step
    incl. the (l-1) steps before its pipeline slot becomes valid) is cancelled
    by a k=0-only weight matrix whose ones-row carries the correction.

Sharding: batch N=65536 split evenly across the 8 NeuronCores.
Measured ~145-175us per core on trn2 (vs ~450us for the naive is_ge +
copy_predicated version); exact (0.0) error vs the fp32 reference.
"""

import numpy as np

import concourse.bacc as bacc
import concourse.mybir as mybir
import concourse.tile as tile
from concourse.bass_utils import run_bass_kernel_spmd

F32 = mybir.dt.float32
F32R = mybir.dt.float32r

N_CORES = 8
N_TOTAL = 65536
N_PER_CORE = N_TOTAL // N_CORES          # 8192
TILE_N = 512                              # samples per PSUM bank (fp32 limit)
N_TILES = N_PER_CORE // TILE_N            # 16
T = 16
N_LAYERS = 6
STEPS = T + N_LAYERS - 1                  # 21 wavefront steps with valid work
# one extra matmul step so the accumulator rows pick up the last s6 spikes
MM_STEPS = STEPS + 1                      # 22

# feature rows of the membrane state (v) / spike rows
ROWS = [18, 18, 18, 9, 10, 2]             # v1..v6
ROW_OFF = np.cumsum([0] + ROWS).tolist()  # [0,18,36,54,63,73,75]
NV = ROW_OFF[-1]                          # 75
K_X = NV                                  # x rows start (75..83)
K_ONE = NV + 9                            # ones row (84)
K_TOT = NV + 9 + 1                        # 85
M_ACC = NV                                # acc cols start (75..76)
M_TOT = NV + 2                            # 77
THRESHOLDS = [20.0, 10.0, 8.0, 8.0, 30.0, 30.0]


def _conv_matrix(w):
    """3x3 SAME conv on a 3x3 image as a dense [Cout*9, Cin*9] matrix.

    Feature index = c*9 + i*3 + j; out[o] = sum_k M[o, k] * in[k].
    """
    co, ci = w.shape[0], w.shape[1]
    m = np.zeros((co * 9, ci * 9), np.float32)
    for o in range(co):
        for c in range(ci):
            for oi in range(3):
                for oj in range(3):
                    for ii in range(3):
                        for ij in range(3):
                            kh, kw = ii - oi + 1, ij - oj + 1
                            if 0 <= kh < 3 and 0 <= kw < 3:
                                m[o * 9 + oi * 3 + oj, c * 9 + ii * 3 + ij] = \
                                    w[o, c, kh, kw]
    return m


def _build_constants(w1, b1, w2, b2, w3, b3, w4, b4, wfc1, wfc2,
                     mode="basic"):
    """Wblk [K_TOT, M_TOT], thr [NV,1], vinit [NV,1] as numpy arrays.

    mode:
      basic       - spike rows carry s in {0,1} (is_ge), reset by copy_predicated
      clamp       - adds a -theta*I diagonal feedback block (spike rows -> own
                    membrane columns); with a per-step clamp v=min(v,theta)
                    this reproduces the hard reset exactly (the clamp pins v
                    to exactly theta at spike time, so subtracting theta on
                    the next step equals reset-to-zero)
      sigma_clamp - clamp feedback plus sigma encoding: spike rows carry
                    sigma = sign(v-theta) in {-1,+1} (computed on the Scalar
                    engine); since s = (sigma+1)/2, all spike-row weights are
                    halved and their row-sums/2 move into the ones-row bias.
                    With rows initialized to -1, inactive layers contribute
                    exactly zero.
    """
    mats = [
        _conv_matrix(w1),                 # 9  -> 18
        _conv_matrix(w2),                 # 18 -> 18
        _conv_matrix(w3),                 # 18 -> 18
        _conv_matrix(w4),                 # 18 -> 9
        np.asarray(wfc1, np.float32),     # 9  -> 10
        np.asarray(wfc2, np.float32),     # 10 -> 2
    ]
    biases = [
        np.repeat(np.asarray(b1, np.float32), 9),
        np.repeat(np.asarray(b2, np.float32), 9),
        np.repeat(np.asarray(b3, np.float32), 9),
        np.repeat(np.asarray(b4, np.float32), 9),
        np.zeros(10, np.float32),
        np.zeros(2, np.float32),
    ]

    wblk = np.zeros((K_TOT, M_TOT), np.float32)
    # layer 1: x rows -> v1 cols
    wblk[K_X:K_X + 9, 0:18] = mats[0].T
    # layers 2..6: spike rows of layer l-1 -> v_l cols
    for l in range(1, 6):
        r0, r1 = ROW_OFF[l - 1], ROW_OFF[l]      # spike rows (prev layer)
        c0, c1 = ROW_OFF[l], ROW_OFF[l + 1]      # v cols (this layer)
        wblk[r0:r1, c0:c1] = mats[l].T
    # s6 rows -> output accumulator cols, scaled by 1/T
    wblk[ROW_OFF[5]:ROW_OFF[6], M_ACC:M_ACC + 2] = np.eye(2, dtype=np.float32) / T
    # ones row -> biases
    for l in range(6):
        wblk[K_ONE, ROW_OFF[l]:ROW_OFF[l + 1]] = biases[l]
    if mode in ("clamp", "sigma_clamp"):
        # spike rows -> own membrane columns: subtract theta on next step
        for l in range(6):
            r0, r1 = ROW_OFF[l], ROW_OFF[l + 1]
            wblk[r0:r1, r0:r1] += -THRESHOLDS[l] * np.eye(r1 - r0,
                                                          dtype=np.float32)
    if mode == "sigma_clamp":
        # s = (sigma+1)/2: halve spike-row weights, move row-sums/2 into bias
        half = wblk[0:NV, :] * 0.5
        wblk[K_ONE, :] += half.sum(axis=0)
        wblk[0:NV, :] = half

    thr = np.zeros((NV, 1), np.float32)
    vinit = np.zeros((NV, 1), np.float32)
    for l in range(6):
        thr[ROW_OFF[l]:ROW_OFF[l + 1], 0] = THRESHOLDS[l]
        # layer l (0-indexed) gets its bias added on l warmup steps (k=0..l-1)
        # before its valid window starts at k=l; cancel them.
        vinit[ROW_OFF[l]:ROW_OFF[l + 1], 0] = -float(l) * biases[l]
    return wblk, thr, vinit


def build_program(n_tiles=N_TILES, repeat=1, elementwise=True,
                  mode="sigma_clamp", span=1024, loop_repeat=0):
    """span: samples per PSUM tile (512 = 1 bank, 1024 = 2 banks)."""
    n_samp = n_tiles * TILE_N
    assert span % TILE_N == 0 and n_samp % span == 0
    n_mm = span // TILE_N                 # matmuls per step per span-tile
    n_stiles = n_samp // span
    nc = bacc.Bacc("TRN2", target_bir_lowering=False, debug=False)

    # 10 rows: 9 pixel rows + a row of ones (bias input), pre-built on host
    xst = nc.dram_tensor("xst", [10, n_samp], F32R, kind="ExternalInput")
    wblk = nc.dram_tensor("wblk", [K_TOT, M_TOT], F32R, kind="ExternalInput")
    # k=0 weights: ones-row additionally carries the warmup-bias cancellation
    wblk0 = nc.dram_tensor("wblk0", [K_TOT, M_TOT], F32R, kind="ExternalInput")
    thr = nc.dram_tensor("thr", [NV, 1], F32, kind="ExternalInput")
    negthr = nc.dram_tensor("negthr", [NV, 1], F32, kind="ExternalInput")
    out = nc.dram_tensor("out", [2, n_samp], F32, kind="ExternalOutput")

    with tile.TileContext(nc) as tc:
        with tc.tile_pool(name="const", bufs=1) as constp, \
             tc.tile_pool(name="rhs", bufs=max(2, 8 // n_mm)) as rhsp, \
             tc.tile_pool(name="res", bufs=4) as resp, \
             tc.tile_pool(name="psum", bufs=max(2, 8 // n_mm),
                          space="PSUM") as psump:

            wblk_t = constp.tile([K_TOT, M_TOT], F32R)
            nc.sync.dma_start(wblk_t[:], wblk[:])
            wblk0_t = constp.tile([K_TOT, M_TOT], F32R)
            nc.sync.dma_start(wblk0_t[:], wblk0[:])
            thr_t = constp.tile([NV, 1], F32)
            nc.sync.dma_start(thr_t[:], thr[:])
            negthr_t = constp.tile([NV, 1], F32)
            nc.sync.dma_start(negthr_t[:], negthr[:])
            zeros_t = constp.tile([NV, TILE_N], F32)
            nc.gpsimd.memset(zeros_t[:], 0.0)

            def tile_body(j):
                rhs = rhsp.tile([K_TOT, span], F32R)
                psum = psump.tile([M_TOT, span], F32)

                # static rows: spike rows start at "no spike" (0 in s
                # encoding, -1 in sigma encoding); x pixels + ones via DMA.
                # (memset has no f32r flavor - write the bits as uint32)
                init_bits = 0xBF800000 if mode == "sigma_clamp" else 0
                nc.gpsimd.memset(rhs[0:NV, :].bitcast(mybir.dt.uint32),
                                 init_bits)
                nc.sync.dma_start(
                    rhs[K_X:K_X + 10, :],
                    xst[:, j * span:(j + 1) * span],
                )

                for k in range(MM_STEPS):
                    # The membrane state lives in PSUM across all steps: the
                    # matmul accumulates onto it (start only at k=0) while
                    # ACT/DVE read/rewrite it between steps.  That
                    # interleaving is serialized by Tile dependency tracking
                    # and is fine on HW (has_written bits persist across
                    # engine writes), but the sim's conservative group guard
                    # must be skipped.
                    w = wblk0_t if k == 0 else wblk_t
                    for m in range(n_mm):
                        nc.tensor.matmul(
                            psum[:, m * TILE_N:(m + 1) * TILE_N],
                            w[:],
                            rhs[:, m * TILE_N:(m + 1) * TILE_N],
                            start=(k == 0),
                            stop=(k == MM_STEPS - 1),
                            skip_group_check=True,
                        )
                    if k < MM_STEPS - 1 and elementwise:
                        # spikes (also feeds next matmul + acc rows)
                        if mode == "sigma_clamp":
                            # sigma = sign(v - theta), on the Scalar engine
                            nc.scalar.activation(
                                rhs[0:NV, :], psum[0:NV, :],
                                mybir.ActivationFunctionType.Sign,
                                bias=negthr_t[:], scale=1.0,
                            )
                        else:
                            nc.vector.tensor_scalar(
                                rhs[0:NV, :], psum[0:NV, :],
                                thr_t[:], None, mybir.AluOpType.is_ge,
                            )
                    if k < MM_STEPS - 2 and elementwise:
                        if mode in ("clamp", "sigma_clamp"):
                            # clamp to theta; with the -theta*I feedback in
                            # Wblk this is an exact hard reset (see above)
                            nc.vector.tensor_scalar(
                                psum[0:NV, :], psum[0:NV, :],
                                thr_t[:], None, mybir.AluOpType.min,
                            )
                        else:
                            # hard reset to zero where spiked (mask viewed as
                            # uint32: 1.0f bits nonzero, 0.0f bits zero)
                            for m in range(n_mm):
                                nc.vector.copy_predicated(
                                    psum[0:NV, m * TILE_N:(m + 1) * TILE_N],
                                    rhs[0:NV, m * TILE_N:(m + 1) * TILE_N]
                                    .bitcast(mybir.dt.uint32),
                                    zeros_t[:],
                                )

                # engines need quadrant-aligned partition bases: copy from
                # partition 64 (13 rows) and DMA out the last two rows.
                res = resp.tile([13, span], F32)
                nc.vector.tensor_copy(res[:], psum[64:M_TOT, :])
                nc.sync.dma_start(
                    out[:, j * span:(j + 1) * span],
                    res[M_ACC - 64:M_TOT - 64, :],
                )

            # timing mode (repeat > 1) statically unrolls the whole
            # computation to amortize away host/axon dispatch overhead
            if loop_repeat:
                # hardware loop around the full computation: cannot be
                # dead-code-eliminated, so wall-clock slope over
                # loop_repeat measures true device time per iteration
                with tc.For_i(0, loop_repeat, 1):
                    for j in range(n_stiles):
                        tile_body(j)
            else:
                for _ in range(repeat):
                    for j in range(n_stiles):
                        tile_body(j)

    nc.compile()
    return nc


# ---- optimized on-device firing check ------------------------------------
# Layout constants.  14 sample groups of 9 pixel rows share each matmul
# column: A-half = PE rows 0..62 (+ ones row 63), B-half = rows 64..126
# (+ ones row 127).  Out partitions 18g+f (7 groups x 18 conv1 features).
# The ones row streams a -tau weight row, so psum = conv1(x) - tau directly
# and the firing test is simply "any psum >= 0".
# Scan balance: DVE (banks 0+2) runs at 1.04ns/col with ~170ns fixed per
# instruction; ACT (bank 1) at 0.83ns/col with ~330ns fixed plus it starts
# one matmul later.  B1C=454 equalizes both engines' finish times.
CHK_B1C = 454            # B-half cols = ACT's bank (7*454 = 3178 samples)
CHK_XCOLS = (N_PER_CORE - 7 * CHK_B1C + 6) // 7        # A-half cols (717)
CHK_A2C = CHK_XCOLS - 512     # A leftover bank cols (205)
CHK_WOFF = CHK_XCOLS     # weight block starts here
CHK_NPACK = CHK_XCOLS + 126   # packed tensor [128, CHK_NPACK]
BF16 = mybir.dt.bfloat16
# check-program element type: fp8e4m3 halves the DMA bytes vs bf16; its
# rounding error is rigorously absorbed into the tau margin (REL/ABS below)
CHK_DT = mybir.dt.float8e4
CHK_REL = {mybir.dt.float8e4: 2.0 ** -4, BF16: 2.0 ** -8}[CHK_DT]
CHK_ABS = {mybir.dt.float8e4: 2.0 ** -10, BF16: 2.0 ** -100}[CHK_DT]


def build_check_program(loop_repeat=0, unroll=1, sb_bufs=4, body_unroll=2):
    """Event-driven fast path: decide on-device whether ANY sample can ever
    produce a layer-1 spike.

    An IF layer with constant per-step drive c fires iff max_t v1(t) =
    16*c >= theta1 for some feature.  conv1's drive c1 = M1 x + b1 is
    constant across time, so the whole network's activity reduces to this
    single check when the zero-input trajectory is silent (host-verified).

    v2 design, tuned against the TRN2 instruction cost model:
      - ONE input DMA: x (bf16, 14 groups of 9 pixel rows block-packed),
        the block-diagonal conv1 weights AND the -tau threshold row ride
        in a single [128, CHK_NPACK] bf16 tensor (HWDGE fixed cost ~2us
        per DMA dominates; N DMAs serialize on the HWDGE).
      - 3 matmuls (512+512+147 cols, one PSUM bank each): 7 samples per
        column, tau folded in via the ones row -> psum = c1_raw - tau.
      - The scan splits across two engines reading DIFFERENT psum banks in
        parallel: DVE reduce_max on banks 0+2, ACT sign+accumulate on bank
        1 (accum == -512 exactly iff every column is silent).
      - ONE tiny output DMA [126, 3] with both engines' verdicts.
      - No gpsimd/Pool usage, no zero-output DMA (the host emits the zeros).
    """
    nc = bacc.Bacc("TRN2", target_bir_lowering=False, debug=False)
    xpk = nc.dram_tensor("xpk", [128, CHK_NPACK], CHK_DT, kind="ExternalInput")
    flagout = nc.dram_tensor("flagout", [126, 3], F32, kind="ExternalOutput")

    with tile.TileContext(nc) as tc:
        with tc.tile_pool(name="const", bufs=1) as constp, \
             tc.tile_pool(name="sb", bufs=sb_bufs) as sb, \
             tc.tile_pool(name="ps", bufs=2, space="PSUM") as ps:
            # hoist the ACT Sign table load off the critical path: a dummy
            # 1-elem activation at t=0 (under the input DMA) loads the
            # function set once; later Sign activations reuse it.
            scratch = constp.tile([1, 2], F32)
            nc.gpsimd.memset(scratch[:], 0.0)
            nc.scalar.activation(scratch[0:1, 1:2], scratch[0:1, 0:1],
                                 mybir.ActivationFunctionType.Sign,
                                 bias=0.0, scale=1.0)

            def check_body(_i):
                xt = sb.tile([128, CHK_NPACK], CHK_DT, name="xt", tag="xt")
                nc.sync.dma_start(xt[:], xpk[:])
                # one PSUM tile per bank so each scan depends only on its
                # own matmul (a shared tile serializes reads behind every
                # write to it)
                ps0 = ps.tile([126, 512], F32, name="ps0", tag="ps0")
                ps1 = ps.tile([126, CHK_B1C], F32, name="ps1", tag="ps1")
                ps2 = ps.tile([126, CHK_A2C], F32, name="ps2", tag="ps2")
                wA = xt[0:64, CHK_WOFF:CHK_WOFF + 126]
                wB = xt[64:128, CHK_WOFF:CHK_WOFF + 126]
                nc.tensor.matmul(ps0[:], wA, xt[0:64, 0:512],
                                 start=True, stop=True, tile_position=(0, 0))
                nc.tensor.matmul(ps1[:], wB, xt[64:128, 0:CHK_B1C],
                                 start=True, stop=True, tile_position=(64, 0))
                nc.tensor.matmul(ps2[:], wA, xt[0:64, 512:CHK_XCOLS],
                                 start=True, stop=True, tile_position=(0, 0))
                flags = sb.tile([126, 3], F32, name="flags", tag="flags")
                nc.vector.reduce_max(flags[:, 0:1], ps0[:],
                                     axis=mybir.AxisListType.X)
                # ACT: sigma = sign(psum) in place, accum = sum(sigma);
                # silent bank <=> accum == -512 exactly
                nc.scalar.activation(ps1[:], ps1[:],
                                     mybir.ActivationFunctionType.Sign,
                                     bias=0.0, scale=1.0,
                                     accum_out=flags[:, 2:3])
                nc.vector.reduce_max(flags[:, 1:2], ps2[:],
                                     axis=mybir.AxisListType.X)
                # flag DMA alternates between the SP and ACT HWDGE rings so
                # SP's stream is back-to-back input DMAs: with both DMAs on
                # SP, iteration k+1's input dispatch queues behind iteration
                # k's output (which waits on k's scans), fully serializing
                # the loop (~2.4us/iter -> ~1.3us/iter)
                eng = nc.scalar if _i % 2 else nc.sync
                eng.dma_start(flagout[:], flags[:])

            if loop_repeat:
                assert loop_repeat % body_unroll == 0, \
                    "loop_repeat must divide body_unroll"
                with tc.For_i(0, loop_repeat // body_unroll, 1):
                    for i in range(body_unroll):
                        check_body(i)
            elif unroll > 1:
                for i in range(unroll):
                    check_body(i)
            else:
                check_body(0)

    nc.compile()
    return nc


def build_check_program_raw():
    """Raw-bass (no TileContext) variant of the check program: hand-rolled
    semaphores instead of Tile's start/drain barrier cascade (~1.1us of the
    single-shot critical path in the TRN2 cost model).

    Protocol (all sems 0 at entry, restored to 0 at exit so repeated NEFF
    executions stay correct):
      SP:   dma xpk->xt (+16 s_dma) ; wait s_scan>=3 ; dma flags->flagout
            (+16 s_out) ; wait s_out>=16 ; clear all sems
      Pool: memset scratch (+1 s_z)
      ACT:  wait s_z ; dummy Sign (hoists the 1.3us act-table load under
            the input DMA) ; wait s_mm>=2 ; Sign+accum on bank1 (+1 s_scan)
      PE:   wait s_dma>=16 ; matmul ps0/ps1/ps2 (+1 s_mm each)
      DVE:  wait s_mm>=1 ; rmax ps0 (+1 s_scan) ; wait s_mm>=3 ; rmax ps2
            (+1 s_scan)
    """
    import concourse.bass as bass
    nc = bacc.Bacc("TRN2", target_bir_lowering=False, debug=False)
    xpk = nc.dram_tensor("xpk", [128, CHK_NPACK], CHK_DT, kind="ExternalInput")
    flagout = nc.dram_tensor("flagout", [126, 3], F32, kind="ExternalOutput")

    with nc.semaphore("s_dma") as s_dma, \
         nc.semaphore("s_mm") as s_mm, \
         nc.semaphore("s_scan") as s_scan, \
         nc.semaphore("s_out") as s_out, \
         nc.sbuf_tensor("xt", [128, CHK_NPACK], CHK_DT) as xt, \
         nc.sbuf_tensor("flags", [126, 3], F32) as flags, \
         nc.sbuf_tensor("scratch", [1, 2], F32) as scratch, \
         nc.psum_tensor("ps0", [126, 512], F32) as ps0, \
         nc.psum_tensor("ps1", [126, CHK_B1C], F32) as ps1, \
         nc.psum_tensor("ps2", [126, CHK_A2C], F32) as ps2:

        # manual BassBlock with a barrier-free exit: for a single-block
        # program the exit all-engine barrier (~280ns after the final DMA
        # wait) only serves block composition; each engine halting at the
        # end of its own drained stream is sufficient
        block = bass.BassBlock(nc, f"chk_{nc.next_id()}", no_gpsimd_drain=True)
        nc.cur_block = block
        if True:

            @block.sync
            def _(sync):
                # bass's preamble re-clears all kernel sems at the start of
                # every NEFF execution, so no explicit restore is needed
                sync.dma_start(xt[:], xpk[:]).then_inc(s_dma, 16)
                sync.wait_ge(s_scan, 4)
                # stall SP until the flag DMA's completion sem fires: ~300ns
                # of tail in the cost model, but guarantees flagout is in
                # DRAM before the NEFF reports done.  (Without it, a runtime
                # that reads outputs immediately at engine-halt could see a
                # stale all-zero flagout -> false fire -> the ~150us dense
                # fallback.  Tile programs always wait; match that.)
                sync.dma_start(flagout[:], flags[:]).then_inc(s_out, 16)
                sync.wait_ge(s_out, 16)

            @block.scalar
            def _(scalar):
                scalar.wait_ge(s_scan, 1)
                scalar.activation(scratch[0:1, 1:2], scratch[0:1, 0:1],
                                  mybir.ActivationFunctionType.Sign,
                                  bias=0.0, scale=1.0)
                scalar.wait_ge(s_mm, 2)
                scalar.activation(ps1[:], ps1[:],
                                  mybir.ActivationFunctionType.Sign,
                                  bias=0.0, scale=1.0,
                                  accum_out=flags[:, 2:3]).then_inc(s_scan, 1)

            @block.tensor
            def _(tensor):
                tensor.wait_ge(s_dma, 16)
                tensor.matmul(ps0[:], xt[0:64, CHK_WOFF:CHK_WOFF + 126],
                              xt[0:64, 0:512], start=True, stop=True,
                              tile_position=(0, 0)).then_inc(s_mm, 1)
                tensor.matmul(ps1[:], xt[64:128, CHK_WOFF:CHK_WOFF + 126],
                              xt[64:128, 0:CHK_B1C], start=True, stop=True,
                              tile_position=(64, 0)).then_inc(s_mm, 1)
                tensor.matmul(ps2[:], xt[0:64, CHK_WOFF:CHK_WOFF + 126],
                              xt[0:64, 512:CHK_XCOLS], start=True, stop=True,
                              tile_position=(0, 0)).then_inc(s_mm, 1)

            @block.vector
            def _(vector):
                # DVE's first instruction: init the ACT warmup scratch (its
                # s_scan inc is guaranteed first since DVE runs in order)
                vector.memset(scratch[:], 0.0).then_inc(s_scan, 1)
                vector.wait_ge(s_mm, 1)
                vector.reduce_max(flags[:, 0:1], ps0[:],
                                  axis=mybir.AxisListType.X).then_inc(s_scan, 1)
                vector.wait_ge(s_mm, 3)
                vector.reduce_max(flags[:, 1:2], ps2[:],
                                  axis=mybir.AxisListType.X).then_inc(s_scan, 1)

        # barrier-free Block exit (mirrors BassBlock.__exit__ minus
        # all_engine_barrier): branch each engine to the end block and
        # drain the non-GpSimd engines
        for engine, last_body in block.last_body.items():
            with nc.body(last_body, parent=nc.cur_bb,
                         allow_existing_parent=True):
                engine.br(block.end_bb)
        nc.switch_bb(block.end_bb)
        for eng_type, eng in nc.engines.items():
            if eng_type == mybir.EngineType.Pool:
                continue
            d = mybir.InstDrain(name=nc.get_next_instruction_name(),
                                ins=[], outs=[], bass_is_fusable=False)
            d.engine = eng_type
            eng.add_instruction(d)
        nc.cur_block = None

    nc.compile()
    return nc


def _check_tau_and_err(w1, b1, maxabs_x):
    """Per-feature device threshold tau and its soundness margin.

    Device flags iff conv1_raw(x)_f >= tau_f for some sample/feature, where
    tau_f = (theta1 - EPS)/16 - b1_f.  EPS rigorously covers the CHK_DT
    rounding of x and M1 (relative CHK_REL, subnormal floor CHK_ABS; the
    M1 term is computed exactly from the actual cast), fp32 accumulation-
    order drift vs the jax fp32 reference conv, and the reference's T
    sequential adds.  tau itself is pre-shifted one rounding bound DOWN so
    its own CHK_DT cast stays conservative.
    """
    npdt = mybir.dt.np(CHK_DT)
    m1 = _conv_matrix(np.asarray(w1, np.float32))          # [18, 9]
    m1c = m1.astype(npdt).astype(np.float32)               # device weights
    b1r = np.repeat(np.asarray(b1, np.float32), 9)         # [18]
    err = (np.abs(m1c).sum(axis=1) * (float(maxabs_x) * CHK_REL + CHK_ABS)
           + np.abs(m1c - m1).sum(axis=1) * float(maxabs_x)
           + 1e-3)                                         # [18]
    tau = (THRESHOLDS[0] / 16.0) - b1r - err               # [18]
    tau_down = tau - (np.abs(tau) * CHK_REL + CHK_ABS) - 1e-6
    return tau_down.astype(np.float32)


def make_check_in_maps(x, w1, b1):
    """Per-core packed [128, CHK_NPACK] CHK_DT inputs for the check program."""
    npdt = mybir.dt.np(CHK_DT)
    xs = np.asarray(x, np.float32).reshape(N_TOTAL, 9)
    maxabs_x = float(np.abs(xs).max())
    tau = _check_tau_and_err(w1, b1, maxabs_x)             # [18]
    m1 = _conv_matrix(np.asarray(w1, np.float32))          # [18, 9]

    # weight block [128, 126]: rows 9g+k -> psum partition 18g+f carries
    # M1[f, k]; ones rows 63/127 carry -tau (repeated per group)
    wblk = np.zeros((128, 126), np.float32)
    for g in range(7):
        wblk[9 * g:9 * g + 9, 18 * g:18 * g + 18] = m1.T
        wblk[64 + 9 * g:64 + 9 * g + 9, 18 * g:18 * g + 18] = m1.T
    wblk[63] = np.tile(-tau, 7)
    wblk[127] = np.tile(-tau, 7)

    in_maps = []
    for core in range(N_CORES):
        shard = xs[core * N_PER_CORE:(core + 1) * N_PER_CORE]   # [8192, 9]
        # A-half: samples 0..4612 as [7, 659, 9]; B-half: samples
        # 4613..8191 (+5 dups) as [7, 512, 9]; B cols 512.. are never read
        # by a matmul - fill with sample 0.
        na = 7 * CHK_XCOLS
        a = shard[np.minimum(np.arange(na), N_PER_CORE - 1)]
        a = a.reshape(7, CHK_XCOLS, 9)
        bidx = np.minimum(na + np.arange(7 * CHK_B1C), N_PER_CORE - 1)
        b = shard[bidx].reshape(7, CHK_B1C, 9)
        pk = np.zeros((128, CHK_NPACK), np.float32)
        pk[0:63, 0:CHK_XCOLS] = a.transpose(0, 2, 1).reshape(63, CHK_XCOLS)
        pk[64:127, 0:CHK_B1C] = b.transpose(0, 2, 1).reshape(63, CHK_B1C)
        pk[64:127, CHK_B1C:CHK_XCOLS] = np.tile(shard[0], 7).reshape(63, 1)
        pk[63, 0:CHK_XCOLS] = 1.0
        pk[127, 0:CHK_XCOLS] = 1.0
        pk[:, CHK_WOFF:] = wblk
        in_maps.append({"xpk": pk.astype(npdt)})
    return in_maps


def check_flags_fire(results):
    """Host-side verdict from the check program's flagout tensors."""
    for r in results:
        f = np.asarray(r["flagout"], np.float32)           # [126, 3]
        if (f[:, 0] >= 0.0).any() or (f[:, 1] >= 0.0).any():
            return True
        if (f[:, 2] > -(CHK_B1C - 0.5)).any():
            return True
    return False


def quiet_zero_input(b1, b2, b3, b4):
    """Host check: with zero input spikes, is every layer silent (with
    margin)?  If layers 1..l-1 are silent a sample's layer-l potential is
    t*b_l, so silence of the bias-only trajectory is checked layer by
    layer.  Margin 1e-2 absorbs any fp32 accumulation drift."""
    for b, thr in zip((np.asarray(b1), np.asarray(b2), np.asarray(b3),
                       np.asarray(b4)), THRESHOLDS[:4]):
        drive = float(np.maximum(np.asarray(b, np.float32), 0.0).max())
        if T * drive >= thr - 1e-2:
            return False
    return True   # fc layers have zero bias in this architecture



_PROGRAM_CACHE = {}


def _get_program():
    if "nc" not in _PROGRAM_CACHE:
        _PROGRAM_CACHE["nc"] = build_program()
    return _PROGRAM_CACHE["nc"]


def _get_check_program():
    if "chk" not in _PROGRAM_CACHE:
        # raw-bass variant: no Tile barrier cascade on the single-shot
        # critical path (the Tile build_check_program remains for the
        # loop_repeat timing mode used by test.py)
        _PROGRAM_CACHE["chk"] = build_check_program_raw()
    return _PROGRAM_CACHE["chk"]


def make_in_maps(x, w1, b1, w2, b2, w3, b3, w4, b4, wfc1, wfc2,
                 mode="sigma_clamp"):
    wblk, thr, vinit = _build_constants(
        np.asarray(w1, np.float32), np.asarray(b1, np.float32),
        np.asarray(w2, np.float32), np.asarray(b2, np.float32),
        np.asarray(w3, np.float32), np.asarray(b3, np.float32),
        np.asarray(w4, np.float32), np.asarray(b4, np.float32),
        np.asarray(wfc1, np.float32), np.asarray(wfc2, np.float32),
        mode=mode)
    wblk0 = wblk.copy()
    wblk0[K_ONE, 0:NV] += vinit[:, 0]
    xs = np.asarray(x, np.float32).reshape(N_TOTAL, 9)
    in_maps = []
    for c in range(N_CORES):
        shard = xs[c * N_PER_CORE:(c + 1) * N_PER_CORE]
        xst = np.ones((10, N_PER_CORE), np.float32)
        xst[0:9] = shard.T
        in_maps.append({
            "xst": xst,
            "wblk": wblk,
            "wblk0": wblk0,
            "thr": thr,
            "negthr": -thr,
        })
    return in_maps


def kernel(x, w1, b1, w2, b2, w3, b3, w4, b4, wfc1, wfc2, T=16, **_):
    assert int(T) == 16, "kernel is specialized for T=16"
    # Event-driven fast path: when the bias-only trajectory is silent
    # (weights-derived, host-checked), network activity reduces to the
    # per-sample layer-1 firing condition 16*c1 >= theta1, checked on
    # device.  If no sample can fire, the output is exactly zero.
    if quiet_zero_input(b1, b2, b3, b4):
        chk = _get_check_program()
        chk_maps = make_check_in_maps(x, w1, b1)
        res = run_bass_kernel_spmd(chk, chk_maps,
                                   core_ids=list(range(N_CORES)))
        # device computed psum = conv1_raw(x) - tau with tau folded into
        # the matmul; fire iff any psum >= 0 (DVE rmax banks 0+2) or the
        # ACT sign-accumulator of bank 1 departs from exactly -512
        if not check_flags_fire(res.results):
            return np.zeros((N_TOTAL, 2), np.float32)
    # exact dense path (any potentially-firing sample, or noisy biases)
    nc = _get_program()
    in_maps = make_in_maps(x, w1, b1, w2, b2, w3, b3, w4, b4, wfc1, wfc2,
                           mode="sigma_clamp")
    res = run_bass_kernel_spmd(nc, in_maps, core_ids=list(range(N_CORES)))
    out = np.empty((N_TOTAL, 2), np.float32)
    for c in range(N_CORES):
        out[c * N_PER_CORE:(c + 1) * N_PER_CORE] = res.results[c]["out"].T
    return out



# revision 31
# speedup vs baseline: 1.0769x; 1.0769x over previous
"""Trainium2 Bass kernel for the CSNN (spiking CNN) problem.

Event-driven fast path: spiking networks are sparse by design, and layer 1
receives a time-constant drive c1 = conv1(x)+b1, so a sample can EVER emit
a layer-1 spike iff max_f 16*c1_f >= theta1.  When the bias-only (zero
input) trajectory is silent (host-checked from the weights), network
activity reduces to a single on-device check (build_check_program_raw):
a hand-scheduled raw-bass program (no Tile barriers) whose critical path
is ONE packed fp8e4m3 input DMA (x block-packed 14 sample groups deep,
conv1 weights AND the -tau threshold row fused into the same [128, 843]
tensor), three block-diagonal matmuls computing psum = conv1_raw(x) - tau
(7 samples per PE column; bank cols 512/454/205 balance the two scan
engines), a two-engine parallel scan
(DVE reduce_max on banks 0+2, ACT Sign+accumulate on bank 1, different
banks so the reads overlap), and ONE [126, 3] flag DMA out.  The silent
case (this problem's data: max 16*c1 = 9.8 vs theta = 20) returns the
exact all-zero output; any potentially-firing sample falls back to the
exact dense wavefront kernel below.  The tau margin rigorously absorbs
fp8 rounding of x/M1/tau (computed from the actual casts + maxabs(x)) and
fp32 accumulation-order drift, so the flag can false-fire (slow but
correct) yet never miss a real spike.  ~7.1us single-shot in the TRN2
cost model vs ~13.3us for the session-1 Tile version; Tile-loop HW
steady-state ~1.8-2.2us/iter (body_unroll=16, alternating HWDGE rings).

Dense path network (per sample, T=16 timesteps, all spatial dims 3x3):
  conv1(1->2) -> IF(20) -> conv2(2->2) -> IF(10) -> conv3(2->2) -> IF(8)
  -> conv4(2->1) -> IF(8) -> fc1(9->10) -> IF(30) -> fc2(10->2) -> IF(30)
  output = mean_t spikes6  [N, 2]

Every conv is a 3x3 SAME conv on a 3x3 image, i.e. a dense linear map on the
9*C flattened features.  The whole per-timestep network is therefore a chain
of six small matmuls plus elementwise integrate-and-fire updates.

Kernel formulation (per core, pure data parallel over the batch):
  - One block-diagonal "mega" weight matrix Wblk [85 x 77] evaluates ALL six
    layers at once in a layer-pipelined (wavefront) schedule: at step k,
    layer l processes timestep t = k - (l-1).  fp32r matmuls (full-rate fp32).
  - rhs tile [85 x 1024]: rows 0..74 = spike rows (aligned with the membrane
    rows in PSUM), rows 75..83 = the 9 input pixels (static), row 84 = ones
    (bias input).  1024 samples span two PSUM banks (2 matmuls per step).
  - Membrane potentials v live in PSUM rows 0..74 and are accumulated by the
    matmul itself (start=False).  Rows 75..76 accumulate the layer-6 spikes
    scaled by 1/T (the final output) across steps - also free via matmul.
  - Default mode sigma_clamp needs only TWO elementwise passes per step, one
    per engine:
      ScalarE:  sigma = sign(v - thr) -> rhs spike rows ({-1,+1}; weights are
                rewired for s=(sigma+1)/2, so -1 rows contribute exactly 0)
      VectorE:  v = min(v, thr) - combined with a -thr*I diagonal feedback
                block in Wblk this is an EXACT hard reset: the clamp pins v
                to exactly thr at spike time, so the next step's -thr*sigma
                feedback zeroes it.
    (Caveat: at an exact fp32 tie v == thr, sign() returns 0, encoding half a
    spike; measure-zero and irrelevant at this problem's threshold margins.)
  - Warmup bias over-accumulation (each layer receives its bias on every step
    incl. the (l-1) steps before its pipeline slot becomes valid) is cancelled
    by a k=0-only weight matrix whose ones-row carries the correction.

Sharding: batch N=65536 split evenly across the 8 NeuronCores.
Measured ~145-175us per core on trn2 (vs ~450us for the naive is_ge +
copy_predicated version); exact (0.0) error vs the fp32 reference.
"""

import numpy as np

import concourse.bacc as bacc
import concourse.mybir as mybir
import concourse.tile as tile
from concourse.bass_utils import run_bass_kernel_spmd

F32 = mybir.dt.float32
F32R = mybir.dt.float32r

N_CORES = 8
N_TOTAL = 65536
N_PER_CORE = N_TOTAL // N_CORES          # 8192
TILE_N = 512                              # samples per PSUM bank (fp32 limit)
N_TILES = N_PER_CORE // TILE_N            # 16
T = 16
N_LAYERS = 6
STEPS = T + N_LAYERS - 1                  # 21 wavefront steps with valid work
# one extra matmul step so the accumulator rows pick up the last s6 spikes
MM_STEPS = STEPS + 1                      # 22

# feature rows of the membrane state (v) / spike rows
ROWS = [18, 18, 18, 9, 10, 2]             # v1..v6
ROW_OFF = np.cumsum([0] + ROWS).tolist()  # [0,18,36,54,63,73,75]
NV = ROW_OFF[-1]                          # 75
K_X = NV                                  # x rows start (75..83)
K_ONE = NV + 9                            # ones row (84)
K_TOT = NV + 9 + 1                        # 85
M_ACC = NV                                # acc cols start (75..76)
M_TOT = NV + 2                            # 77
THRESHOLDS = [20.0, 10.0, 8.0, 8.0, 30.0, 30.0]


def _conv_matrix(w):
    """3x3 SAME conv on a 3x3 image as a dense [Cout*9, Cin*9] matrix.

    Feature index = c*9 + i*3 + j; out[o] = sum_k M[o, k] * in[k].
    """
    co, ci = w.shape[0], w.shape[1]
    m = np.zeros((co * 9, ci * 9), np.float32)
    for o in range(co):
        for c in range(ci):
            for oi in range(3):
                for oj in range(3):
                    for ii in range(3):
                        for ij in range(3):
                            kh, kw = ii - oi + 1, ij - oj + 1
                            if 0 <= kh < 3 and 0 <= kw < 3:
                                m[o * 9 + oi * 3 + oj, c * 9 + ii * 3 + ij] = \
                                    w[o, c, kh, kw]
    return m


def _build_constants(w1, b1, w2, b2, w3, b3, w4, b4, wfc1, wfc2,
                     mode="basic"):
    """Wblk [K_TOT, M_TOT], thr [NV,1], vinit [NV,1] as numpy arrays.

    mode:
      basic       - spike rows carry s in {0,1} (is_ge), reset by copy_predicated
      clamp       - adds a -theta*I diagonal feedback block (spike rows -> own
                    membrane columns); with a per-step clamp v=min(v,theta)
                    this reproduces the hard reset exactly (the clamp pins v
                    to exactly theta at spike time, so subtracting theta on
                    the next step equals reset-to-zero)
      sigma_clamp - clamp feedback plus sigma encoding: spike rows carry
                    sigma = sign(v-theta) in {-1,+1} (computed on the Scalar
                    engine); since s = (sigma+1)/2, all spike-row weights are
                    halved and their row-sums/2 move into the ones-row bias.
                    With rows initialized to -1, inactive layers contribute
                    exactly zero.
    """
    mats = [
        _conv_matrix(w1),                 # 9  -> 18
        _conv_matrix(w2),                 # 18 -> 18
        _conv_matrix(w3),                 # 18 -> 18
        _conv_matrix(w4),                 # 18 -> 9
        np.asarray(wfc1, np.float32),     # 9  -> 10
        np.asarray(wfc2, np.float32),     # 10 -> 2
    ]
    biases = [
        np.repeat(np.asarray(b1, np.float32), 9),
        np.repeat(np.asarray(b2, np.float32), 9),
        np.repeat(np.asarray(b3, np.float32), 9),
        np.repeat(np.asarray(b4, np.float32), 9),
        np.zeros(10, np.float32),
        np.zeros(2, np.float32),
    ]

    wblk = np.zeros((K_TOT, M_TOT), np.float32)
    # layer 1: x rows -> v1 cols
    wblk[K_X:K_X + 9, 0:18] = mats[0].T
    # layers 2..6: spike rows of layer l-1 -> v_l cols
    for l in range(1, 6):
        r0, r1 = ROW_OFF[l - 1], ROW_OFF[l]      # spike rows (prev layer)
        c0, c1 = ROW_OFF[l], ROW_OFF[l + 1]      # v cols (this layer)
        wblk[r0:r1, c0:c1] = mats[l].T
    # s6 rows -> output accumulator cols, scaled by 1/T
    wblk[ROW_OFF[5]:ROW_OFF[6], M_ACC:M_ACC + 2] = np.eye(2, dtype=np.float32) / T
    # ones row -> biases
    for l in range(6):
        wblk[K_ONE, ROW_OFF[l]:ROW_OFF[l + 1]] = biases[l]
    if mode in ("clamp", "sigma_clamp"):
        # spike rows -> own membrane columns: subtract theta on next step
        for l in range(6):
            r0, r1 = ROW_OFF[l], ROW_OFF[l + 1]
            wblk[r0:r1, r0:r1] += -THRESHOLDS[l] * np.eye(r1 - r0,
                                                          dtype=np.float32)
    if mode == "sigma_clamp":
        # s = (sigma+1)/2: halve spike-row weights, move row-sums/2 into bias
        half = wblk[0:NV, :] * 0.5
        wblk[K_ONE, :] += half.sum(axis=0)
        wblk[0:NV, :] = half

    thr = np.zeros((NV, 1), np.float32)
    vinit = np.zeros((NV, 1), np.float32)
    for l in range(6):
        thr[ROW_OFF[l]:ROW_OFF[l + 1], 0] = THRESHOLDS[l]
        # layer l (0-indexed) gets its bias added on l warmup steps (k=0..l-1)
        # before its valid window starts at k=l; cancel them.
        vinit[ROW_OFF[l]:ROW_OFF[l + 1], 0] = -float(l) * biases[l]
    return wblk, thr, vinit


def build_program(n_tiles=N_TILES, repeat=1, elementwise=True,
                  mode="sigma_clamp", span=1024, loop_repeat=0):
    """span: samples per PSUM tile (512 = 1 bank, 1024 = 2 banks)."""
    n_samp = n_tiles * TILE_N
    assert span % TILE_N == 0 and n_samp % span == 0
    n_mm = span // TILE_N                 # matmuls per step per span-tile
    n_stiles = n_samp // span
    nc = bacc.Bacc("TRN2", target_bir_lowering=False, debug=False)

    # 10 rows: 9 pixel rows + a row of ones (bias input), pre-built on host
    xst = nc.dram_tensor("xst", [10, n_samp], F32R, kind="ExternalInput")
    wblk = nc.dram_tensor("wblk", [K_TOT, M_TOT], F32R, kind="ExternalInput")
    # k=0 weights: ones-row additionally carries the warmup-bias cancellation
    wblk0 = nc.dram_tensor("wblk0", [K_TOT, M_TOT], F32R, kind="ExternalInput")
    thr = nc.dram_tensor("thr", [NV, 1], F32, kind="ExternalInput")
    negthr = nc.dram_tensor("negthr", [NV, 1], F32, kind="ExternalInput")
    out = nc.dram_tensor("out", [2, n_samp], F32, kind="ExternalOutput")

    with tile.TileContext(nc) as tc:
        with tc.tile_pool(name="const", bufs=1) as constp, \
             tc.tile_pool(name="rhs", bufs=max(2, 8 // n_mm)) as rhsp, \
             tc.tile_pool(name="res", bufs=4) as resp, \
             tc.tile_pool(name="psum", bufs=max(2, 8 // n_mm),
                          space="PSUM") as psump:

            wblk_t = constp.tile([K_TOT, M_TOT], F32R)
            nc.sync.dma_start(wblk_t[:], wblk[:])
            wblk0_t = constp.tile([K_TOT, M_TOT], F32R)
            nc.sync.dma_start(wblk0_t[:], wblk0[:])
            thr_t = constp.tile([NV, 1], F32)
            nc.sync.dma_start(thr_t[:], thr[:])
            negthr_t = constp.tile([NV, 1], F32)
            nc.sync.dma_start(negthr_t[:], negthr[:])
            zeros_t = constp.tile([NV, TILE_N], F32)
            nc.gpsimd.memset(zeros_t[:], 0.0)

            def tile_body(j):
                rhs = rhsp.tile([K_TOT, span], F32R)
                psum = psump.tile([M_TOT, span], F32)

                # static rows: spike rows start at "no spike" (0 in s
                # encoding, -1 in sigma encoding); x pixels + ones via DMA.
                # (memset has no f32r flavor - write the bits as uint32)
                init_bits = 0xBF800000 if mode == "sigma_clamp" else 0
                nc.gpsimd.memset(rhs[0:NV, :].bitcast(mybir.dt.uint32),
                                 init_bits)
                nc.sync.dma_start(
                    rhs[K_X:K_X + 10, :],
                    xst[:, j * span:(j + 1) * span],
                )

                for k in range(MM_STEPS):
                    # The membrane state lives in PSUM across all steps: the
                    # matmul accumulates onto it (start only at k=0) while
                    # ACT/DVE read/rewrite it between steps.  That
                    # interleaving is serialized by Tile dependency tracking
                    # and is fine on HW (has_written bits persist across
                    # engine writes), but the sim's conservative group guard
                    # must be skipped.
                    w = wblk0_t if k == 0 else wblk_t
                    for m in range(n_mm):
                        nc.tensor.matmul(
                            psum[:, m * TILE_N:(m + 1) * TILE_N],
                            w[:],
                            rhs[:, m * TILE_N:(m + 1) * TILE_N],
                            start=(k == 0),
                            stop=(k == MM_STEPS - 1),
                            skip_group_check=True,
                        )
                    if k < MM_STEPS - 1 and elementwise:
                        # spikes (also feeds next matmul + acc rows)
                        if mode == "sigma_clamp":
                            # sigma = sign(v - theta), on the Scalar engine
                            nc.scalar.activation(
                                rhs[0:NV, :], psum[0:NV, :],
                                mybir.ActivationFunctionType.Sign,
                                bias=negthr_t[:], scale=1.0,
                            )
                        else:
                            nc.vector.tensor_scalar(
                                rhs[0:NV, :], psum[0:NV, :],
                                thr_t[:], None, mybir.AluOpType.is_ge,
                            )
                    if k < MM_STEPS - 2 and elementwise:
                        if mode in ("clamp", "sigma_clamp"):
                            # clamp to theta; with the -theta*I feedback in
                            # Wblk this is an exact hard reset (see above)
                            nc.vector.tensor_scalar(
                                psum[0:NV, :], psum[0:NV, :],
                                thr_t[:], None, mybir.AluOpType.min,
                            )
                        else:
                            # hard reset to zero where spiked (mask viewed as
                            # uint32: 1.0f bits nonzero, 0.0f bits zero)
                            for m in range(n_mm):
                                nc.vector.copy_predicated(
                                    psum[0:NV, m * TILE_N:(m + 1) * TILE_N],
                                    rhs[0:NV, m * TILE_N:(m + 1) * TILE_N]
                                    .bitcast(mybir.dt.uint32),
                                    zeros_t[:],
                                )

                # engines need quadrant-aligned partition bases: copy from
                # partition 64 (13 rows) and DMA out the last two rows.
                res = resp.tile([13, span], F32)
                nc.vector.tensor_copy(res[:], psum[64:M_TOT, :])
                nc.sync.dma_start(
                    out[:, j * span:(j + 1) * span],
                    res[M_ACC - 64:M_TOT - 64, :],
                )

            # timing mode (repeat > 1) statically unrolls the whole
            # computation to amortize away host/axon dispatch overhead
            if loop_repeat:
                # hardware loop around the full computation: cannot be
                # dead-code-eliminated, so wall-clock slope over
                # loop_repeat measures true device time per iteration
                with tc.For_i(0, loop_repeat, 1):
                    for j in range(n_stiles):
                        tile_body(j)
            else:
                for _ in range(repeat):
                    for j in range(n_stiles):
                        tile_body(j)

    nc.compile()
    return nc


# ---- optimized on-device firing check ------------------------------------
# Layout constants.  14 sample groups of 9 pixel rows share each matmul
# column: A-half = PE rows 0..62 (+ ones row 63), B-half = rows 64..126
# (+ ones row 127).  Out partitions 18g+f (7 groups x 18 conv1 features).
# The ones row streams a -tau weight row, so psum = conv1(x) - tau directly
# and the firing test is simply "any psum >= 0".
# Scan balance: DVE (banks 0+2) runs at 1.04ns/col with ~170ns fixed per
# instruction; ACT (bank 1) at 0.83ns/col with ~330ns fixed plus it starts
# one matmul later.  B1C=454 equalizes both engines' finish times.
CHK_B1C = 454            # B-half cols = ACT's bank (7*454 = 3178 samples)
CHK_XCOLS = (N_PER_CORE - 7 * CHK_B1C + 6) // 7        # A-half cols (717)
CHK_A2C = CHK_XCOLS - 512     # A leftover bank cols (205)
CHK_WOFF = CHK_XCOLS     # weight block starts here
CHK_NPACK = CHK_XCOLS + 126   # packed tensor [128, CHK_NPACK]
BF16 = mybir.dt.bfloat16
# check-program element type: fp8e4m3 halves the DMA bytes vs bf16; its
# rounding error is rigorously absorbed into the tau margin (REL/ABS below)
CHK_DT = mybir.dt.float8e4
CHK_REL = {mybir.dt.float8e4: 2.0 ** -4, BF16: 2.0 ** -8}[CHK_DT]
CHK_ABS = {mybir.dt.float8e4: 2.0 ** -10, BF16: 2.0 ** -100}[CHK_DT]


def build_check_program(loop_repeat=0, unroll=1, sb_bufs=4, body_unroll=2):
    """Event-driven fast path: decide on-device whether ANY sample can ever
    produce a layer-1 spike.

    An IF layer with constant per-step drive c fires iff max_t v1(t) =
    16*c >= theta1 for some feature.  conv1's drive c1 = M1 x + b1 is
    constant across time, so the whole network's activity reduces to this
    single check when the zero-input trajectory is silent (host-verified).

    v2 design, tuned against the TRN2 instruction cost model:
      - ONE input DMA: x (bf16, 14 groups of 9 pixel rows block-packed),
        the block-diagonal conv1 weights AND the -tau threshold row ride
        in a single [128, CHK_NPACK] bf16 tensor (HWDGE fixed cost ~2us
        per DMA dominates; N DMAs serialize on the HWDGE).
      - 3 matmuls (512+512+147 cols, one PSUM bank each): 7 samples per
        column, tau folded in via the ones row -> psum = c1_raw - tau.
      - The scan splits across two engines reading DIFFERENT psum banks in
        parallel: DVE reduce_max on banks 0+2, ACT sign+accumulate on bank
        1 (accum == -512 exactly iff every column is silent).
      - ONE tiny output DMA [126, 3] with both engines' verdicts.
      - No gpsimd/Pool usage, no zero-output DMA (the host emits the zeros).
    """
    nc = bacc.Bacc("TRN2", target_bir_lowering=False, debug=False)
    xpk = nc.dram_tensor("xpk", [128, CHK_NPACK], CHK_DT, kind="ExternalInput")
    flagout = nc.dram_tensor("flagout", [126, 3], F32, kind="ExternalOutput")

    with tile.TileContext(nc) as tc:
        with tc.tile_pool(name="const", bufs=1) as constp, \
             tc.tile_pool(name="sb", bufs=sb_bufs) as sb, \
             tc.tile_pool(name="ps", bufs=2, space="PSUM") as ps:
            # hoist the ACT Sign table load off the critical path: a dummy
            # 1-elem activation at t=0 (under the input DMA) loads the
            # function set once; later Sign activations reuse it.
            scratch = constp.tile([1, 2], F32)
            nc.gpsimd.memset(scratch[:], 0.0)
            nc.scalar.activation(scratch[0:1, 1:2], scratch[0:1, 0:1],
                                 mybir.ActivationFunctionType.Sign,
                                 bias=0.0, scale=1.0)

            def check_body(_i):
                xt = sb.tile([128, CHK_NPACK], CHK_DT, name="xt", tag="xt")
                nc.sync.dma_start(xt[:], xpk[:])
                # one PSUM tile per bank so each scan depends only on its
                # own matmul (a shared tile serializes reads behind every
                # write to it)
                ps0 = ps.tile([126, 512], F32, name="ps0", tag="ps0")
                ps1 = ps.tile([126, CHK_B1C], F32, name="ps1", tag="ps1")
                ps2 = ps.tile([126, CHK_A2C], F32, name="ps2", tag="ps2")
                wA = xt[0:64, CHK_WOFF:CHK_WOFF + 126]
                wB = xt[64:128, CHK_WOFF:CHK_WOFF + 126]
                nc.tensor.matmul(ps0[:], wA, xt[0:64, 0:512],
                                 start=True, stop=True, tile_position=(0, 0))
                nc.tensor.matmul(ps1[:], wB, xt[64:128, 0:CHK_B1C],
                                 start=True, stop=True, tile_position=(64, 0))
                nc.tensor.matmul(ps2[:], wA, xt[0:64, 512:CHK_XCOLS],
                                 start=True, stop=True, tile_position=(0, 0))
                flags = sb.tile([126, 3], F32, name="flags", tag="flags")
                nc.vector.reduce_max(flags[:, 0:1], ps0[:],
                                     axis=mybir.AxisListType.X)
                # ACT: sigma = sign(psum) in place, accum = sum(sigma);
                # silent bank <=> accum == -512 exactly
                nc.scalar.activation(ps1[:], ps1[:],
                                     mybir.ActivationFunctionType.Sign,
                                     bias=0.0, scale=1.0,
                                     accum_out=flags[:, 2:3])
                nc.vector.reduce_max(flags[:, 1:2], ps2[:],
                                     axis=mybir.AxisListType.X)
                # flag DMA alternates between the SP and ACT HWDGE rings so
                # SP's stream is back-to-back input DMAs: with both DMAs on
                # SP, iteration k+1's input dispatch queues behind iteration
                # k's output (which waits on k's scans), fully serializing
                # the loop (~2.4us/iter -> ~1.3us/iter)
                eng = nc.scalar if _i % 2 else nc.sync
                eng.dma_start(flagout[:], flags[:])

            if loop_repeat:
                assert loop_repeat % body_unroll == 0, \
                    "loop_repeat must divide body_unroll"
                with tc.For_i(0, loop_repeat // body_unroll, 1):
                    for i in range(body_unroll):
                        check_body(i)
            elif unroll > 1:
                for i in range(unroll):
                    check_body(i)
            else:
                check_body(0)

    nc.compile()
    return nc


def build_check_program_raw():
    """Raw-bass (no TileContext) variant of the check program: hand-rolled
    semaphores instead of Tile's start/drain barrier cascade (~1.1us of the
    single-shot critical path in the TRN2 cost model).

    Protocol (all sems 0 at entry, restored to 0 at exit so repeated NEFF
    executions stay correct):
      SP:   dma xpk->xt (+16 s_dma) ; wait s_scan>=3 ; dma flags->flagout
            (+16 s_out) ; wait s_out>=16 ; clear all sems
      Pool: memset scratch (+1 s_z)
      ACT:  wait s_z ; dummy Sign (hoists the 1.3us act-table load under
            the input DMA) ; wait s_mm>=2 ; Sign+accum on bank1 (+1 s_scan)
      PE:   wait s_dma>=16 ; matmul ps0/ps1/ps2 (+1 s_mm each)
      DVE:  wait s_mm>=1 ; rmax ps0 (+1 s_scan) ; wait s_mm>=3 ; rmax ps2
            (+1 s_scan)
    """
    import concourse.bass as bass
    nc = bacc.Bacc("TRN2", target_bir_lowering=False, debug=False)
    xpk = nc.dram_tensor("xpk", [128, CHK_NPACK], CHK_DT, kind="ExternalInput")
    flagout = nc.dram_tensor("flagout", [126, 3], F32, kind="ExternalOutput")

    with nc.semaphore("s_dma") as s_dma, \
         nc.semaphore("s_mm") as s_mm, \
         nc.semaphore("s_scan") as s_scan, \
         nc.semaphore("s_out") as s_out, \
         nc.sbuf_tensor("xt", [128, CHK_NPACK], CHK_DT) as xt, \
         nc.sbuf_tensor("flags", [126, 3], F32) as flags, \
         nc.sbuf_tensor("scratch", [1, 2], F32) as scratch, \
         nc.psum_tensor("ps0", [126, 512], F32) as ps0, \
         nc.psum_tensor("ps1", [126, CHK_B1C], F32) as ps1, \
         nc.psum_tensor("ps2", [126, CHK_A2C], F32) as ps2:

        # manual BassBlock with a barrier-free exit: for a single-block
        # program the exit all-engine barrier (~280ns after the final DMA
        # wait) only serves block composition; each engine halting at the
        # end of its own drained stream is sufficient
        block = bass.BassBlock(nc, f"chk_{nc.next_id()}", no_gpsimd_drain=True)
        nc.cur_block = block
        if True:

            @block.sync
            def _(sync):
                # bass's preamble re-clears all kernel sems at the start of
                # every NEFF execution, so no explicit restore is needed
                sync.dma_start(xt[:], xpk[:]).then_inc(s_dma, 16)
                sync.wait_ge(s_scan, 4)
                # stall SP until the flag DMA's completion sem fires: ~300ns
                # of tail in the cost model, but guarantees flagout is in
                # DRAM before the NEFF reports done.  (Without it, a runtime
                # that reads outputs immediately at engine-halt could see a
                # stale all-zero flagout -> false fire -> the ~150us dense
                # fallback.  Tile programs always wait; match that.)
                sync.dma_start(flagout[:], flags[:]).then_inc(s_out, 16)
                sync.wait_ge(s_out, 16)

            @block.scalar
            def _(scalar):
                scalar.wait_ge(s_scan, 1)
                scalar.activation(scratch[0:1, 1:2], scratch[0:1, 0:1],
                                  mybir.ActivationFunctionType.Sign,
                                  bias=0.0, scale=1.0)
                scalar.wait_ge(s_mm, 2)
                scalar.activation(ps1[:], ps1[:],
                                  mybir.ActivationFunctionType.Sign,
                                  bias=0.0, scale=1.0,
                                  accum_out=flags[:, 2:3]).then_inc(s_scan, 1)

            @block.tensor
            def _(tensor):
                tensor.wait_ge(s_dma, 16)
                tensor.matmul(ps0[:], xt[0:64, CHK_WOFF:CHK_WOFF + 126],
                              xt[0:64, 0:512], start=True, stop=True,
                              tile_position=(0, 0)).then_inc(s_mm, 1)
                tensor.matmul(ps1[:], xt[64:128, CHK_WOFF:CHK_WOFF + 126],
                              xt[64:128, 0:CHK_B1C], start=True, stop=True,
                              tile_position=(64, 0)).then_inc(s_mm, 1)
                tensor.matmul(ps2[:], xt[0:64, CHK_WOFF:CHK_WOFF + 126],
                              xt[0:64, 512:CHK_XCOLS], start=True, stop=True,
                              tile_position=(0, 0)).then_inc(s_mm, 1)

            @block.vector
            def _(vector):
                # DVE's first instruction: init the ACT warmup scratch (its
                # s_scan inc is guaranteed first since DVE runs in order)
                vector.memset(scratch[:], 0.0).then_inc(s_scan, 1)
                vector.wait_ge(s_mm, 1)
                vector.reduce_max(flags[:, 0:1], ps0[:],
                                  axis=mybir.AxisListType.X).then_inc(s_scan, 1)
                vector.wait_ge(s_mm, 3)
                vector.reduce_max(flags[:, 1:2], ps2[:],
                                  axis=mybir.AxisListType.X).then_inc(s_scan, 1)

        # barrier-free Block exit (mirrors BassBlock.__exit__ minus
        # all_engine_barrier): branch each engine to the end block and
        # drain the non-GpSimd engines
        for engine, last_body in block.last_body.items():
            with nc.body(last_body, parent=nc.cur_bb,
                         allow_existing_parent=True):
                engine.br(block.end_bb)
        nc.switch_bb(block.end_bb)
        for eng_type, eng in nc.engines.items():
            if eng_type == mybir.EngineType.Pool:
                continue
            d = mybir.InstDrain(name=nc.get_next_instruction_name(),
                                ins=[], outs=[], bass_is_fusable=False)
            d.engine = eng_type
            eng.add_instruction(d)
        nc.cur_block = None

    nc.compile()
    return nc


def _check_tau_and_err(w1, b1, maxabs_x):
    """Per-feature device threshold tau and its soundness margin.

    Device flags iff conv1_raw(x)_f >= tau_f for some sample/feature, where
    tau_f = (theta1 - EPS)/16 - b1_f.  EPS rigorously covers the CHK_DT
    rounding of x and M1 (relative CHK_REL, subnormal floor CHK_ABS; the
    M1 term is computed exactly from the actual cast), fp32 accumulation-
    order drift vs the jax fp32 reference conv, and the reference's T
    sequential adds.  tau itself is pre-shifted one rounding bound DOWN so
    its own CHK_DT cast stays conservative.
    """
    npdt = mybir.dt.np(CHK_DT)
    m1 = _conv_matrix(np.asarray(w1, np.float32))          # [18, 9]
    m1c = m1.astype(npdt).astype(np.float32)               # device weights
    b1r = np.repeat(np.asarray(b1, np.float32), 9)         # [18]
    err = (np.abs(m1c).sum(axis=1) * (float(maxabs_x) * CHK_REL + CHK_ABS)
           + np.abs(m1c - m1).sum(axis=1) * float(maxabs_x)
           + 1e-3)                                         # [18]
    tau = (THRESHOLDS[0] / 16.0) - b1r - err               # [18]
    tau_down = tau - (np.abs(tau) * CHK_REL + CHK_ABS) - 1e-6
    return tau_down.astype(np.float32)


def make_check_in_maps(x, w1, b1):
    """Per-core packed [128, CHK_NPACK] CHK_DT inputs for the check program."""
    npdt = mybir.dt.np(CHK_DT)
    xs = np.asarray(x, np.float32).reshape(N_TOTAL, 9)
    maxabs_x = float(np.abs(xs).max())
    tau = _check_tau_and_err(w1, b1, maxabs_x)             # [18]
    m1 = _conv_matrix(np.asarray(w1, np.float32))          # [18, 9]

    # weight block [128, 126]: rows 9g+k -> psum partition 18g+f carries
    # M1[f, k]; ones rows 63/127 carry -tau (repeated per group)
    wblk = np.zeros((128, 126), np.float32)
    for g in range(7):
        wblk[9 * g:9 * g + 9, 18 * g:18 * g + 18] = m1.T
        wblk[64 + 9 * g:64 + 9 * g + 9, 18 * g:18 * g + 18] = m1.T
    wblk[63] = np.tile(-tau, 7)
    wblk[127] = np.tile(-tau, 7)

    in_maps = []
    for core in range(N_CORES):
        shard = xs[core * N_PER_CORE:(core + 1) * N_PER_CORE]   # [8192, 9]
        # A-half: samples 0..4612 as [7, 659, 9]; B-half: samples
        # 4613..8191 (+5 dups) as [7, 512, 9]; B cols 512.. are never read
        # by a matmul - fill with sample 0.
        na = 7 * CHK_XCOLS
        a = shard[np.minimum(np.arange(na), N_PER_CORE - 1)]
        a = a.reshape(7, CHK_XCOLS, 9)
        bidx = np.minimum(na + np.arange(7 * CHK_B1C), N_PER_CORE - 1)
        b = shard[bidx].reshape(7, CHK_B1C, 9)
        pk = np.zeros((128, CHK_NPACK), np.float32)
        pk[0:63, 0:CHK_XCOLS] = a.transpose(0, 2, 1).reshape(63, CHK_XCOLS)
        pk[64:127, 0:CHK_B1C] = b.transpose(0, 2, 1).reshape(63, CHK_B1C)
        pk[64:127, CHK_B1C:CHK_XCOLS] = np.tile(shard[0], 7).reshape(63, 1)
        pk[63, 0:CHK_XCOLS] = 1.0
        pk[127, 0:CHK_XCOLS] = 1.0
        pk[:, CHK_WOFF:] = wblk
        in_maps.append({"xpk": pk.astype(npdt)})
    return in_maps


def check_flags_fire(results):
    """Host-side verdict from the check program's flagout tensors."""
    for r in results:
        f = np.asarray(r["flagout"], np.float32)           # [126, 3]
        if (f[:, 0] >= 0.0).any() or (f[:, 1] >= 0.0).any():
            return True
        if (f[:, 2] > -(CHK_B1C - 0.5)).any():
            return True
    return False


def quiet_zero_input(b1, b2, b3, b4):
    """Host check: with zero input spikes, is every layer silent (with
    margin)?  If layers 1..l-1 are silent a sample's layer-l potential is
    t*b_l, so silence of the bias-only trajectory is checked layer by
    layer.  Margin 1e-2 absorbs any fp32 accumulation drift."""
    for b, thr in zip((np.asarray(b1), np.asarray(b2), np.asarray(b3),
                       np.asarray(b4)), THRESHOLDS[:4]):
        drive = float(np.maximum(np.asarray(b, np.float32), 0.0).max())
        if T * drive >= thr - 1e-2:
            return False
    return True   # fc layers have zero bias in this architecture



_PROGRAM_CACHE = {}


def _get_program():
    if "nc" not in _PROGRAM_CACHE:
        _PROGRAM_CACHE["nc"] = build_program()
    return _PROGRAM_CACHE["nc"]


def _get_check_program():
    if "chk" not in _PROGRAM_CACHE:
        # raw-bass variant: no Tile barrier cascade on the single-shot
        # critical path (the Tile build_check_program remains for the
        # loop_repeat timing mode used by test.py)
        _PROGRAM_CACHE["chk"] = build_check_program_raw()
    return _PROGRAM_CACHE["chk"]


def make_in_maps(x, w1, b1, w2, b2, w3, b3, w4, b4, wfc1, wfc2,
                 mode="sigma_clamp"):
    wblk, thr, vinit = _build_constants(
        np.asarray(w1, np.float32), np.asarray(b1, np.float32),
        np.asarray(w2, np.float32), np.asarray(b2, np.float32),
        np.asarray(w3, np.float32), np.asarray(b3, np.float32),
        np.asarray(w4, np.float32), np.asarray(b4, np.float32),
        np.asarray(wfc1, np.float32), np.asarray(wfc2, np.float32),
        mode=mode)
    wblk0 = wblk.copy()
    wblk0[K_ONE, 0:NV] += vinit[:, 0]
    xs = np.asarray(x, np.float32).reshape(N_TOTAL, 9)
    in_maps = []
    for c in range(N_CORES):
        shard = xs[c * N_PER_CORE:(c + 1) * N_PER_CORE]
        xst = np.ones((10, N_PER_CORE), np.float32)
        xst[0:9] = shard.T
        in_maps.append({
            "xst": xst,
            "wblk": wblk,
            "wblk0": wblk0,
            "thr": thr,
            "negthr": -thr,
        })
    return in_maps


def kernel(x, w1, b1, w2, b2, w3, b3, w4, b4, wfc1, wfc2, T=16, **_):
    assert int(T) == 16, "kernel is specialized for T=16"
    # Event-driven fast path: when the bias-only trajectory is silent
    # (weights-derived, host-checked), network activity reduces to the
    # per-sample layer-1 firing condition 16*c1 >= theta1, checked on
    # device.  If no sample can fire, the output is exactly zero.
    if quiet_zero_input(b1, b2, b3, b4):
        chk = _get_check_program()
        chk_maps = make_check_in_maps(x, w1, b1)
        res = run_bass_kernel_spmd(chk, chk_maps,
                                   core_ids=list(range(N_CORES)))
        # device computed psum = conv1_raw(x) - tau with tau folded into
        # the matmul; fire iff any psum >= 0 (DVE rmax banks 0+2) or the
        # ACT sign-accumulator of bank 1 departs from exactly -512
        if not check_flags_fire(res.results):
            return np.zeros((N_TOTAL, 2), np.float32)
    # exact dense path (any potentially-firing sample, or noisy biases)
    nc = _get_program()
    in_maps = make_in_maps(x, w1, b1, w2, b2, w3, b3, w4, b4, wfc1, wfc2,
                           mode="sigma_clamp")
    res = run_bass_kernel_spmd(nc, in_maps, core_ids=list(range(N_CORES)))
    out = np.empty((N_TOTAL, 2), np.float32)
    for c in range(N_CORES):
        out[c * N_PER_CORE:(c + 1) * N_PER_CORE] = res.results[c]["out"].T
    return out



# revision 32
# speedup vs baseline: 1.2935x; 1.2012x over previous
"""Trainium2 Bass kernel for the CSNN (spiking CNN) problem.

Event-driven fast path: spiking networks are sparse by design, and layer 1
receives a time-constant drive c1 = conv1(x)+b1, so a sample can EVER emit
a layer-1 spike iff max_f 16*c1_f >= theta1.  When the bias-only (zero
input) trajectory is silent (host-checked from the weights), network
activity reduces to a single on-device check (build_check_program_raw):
a hand-scheduled raw-bass program (no Tile barriers) whose critical path
is ONE packed fp8e4m3 input DMA (x block-packed 14 sample groups deep,
conv1 weights AND the -tau threshold row fused into the same [128, 843]
tensor), three block-diagonal matmuls computing psum = conv1_raw(x) - tau
(7 samples per PE column; bank cols 512/454/205 balance the two scan
engines), a two-engine parallel scan
(DVE reduce_max on banks 0+2, ACT Sign+accumulate on bank 1, different
banks so the reads overlap), and ONE [126, 3] flag DMA out.  The silent
case (this problem's data: max 16*c1 = 9.8 vs theta = 20) returns the
exact all-zero output; any potentially-firing sample falls back to the
exact dense wavefront kernel below.  The tau margin rigorously absorbs
fp8 rounding of x/M1/tau (computed from the actual casts + maxabs(x)) and
fp32 accumulation-order drift, so the flag can false-fire (slow but
correct) yet never miss a real spike.  ~7.1us single-shot in the TRN2
cost model vs ~13.3us for the session-1 Tile version; Tile-loop HW
steady-state ~1.8-2.2us/iter (body_unroll=16, alternating HWDGE rings).

Dense path network (per sample, T=16 timesteps, all spatial dims 3x3):
  conv1(1->2) -> IF(20) -> conv2(2->2) -> IF(10) -> conv3(2->2) -> IF(8)
  -> conv4(2->1) -> IF(8) -> fc1(9->10) -> IF(30) -> fc2(10->2) -> IF(30)
  output = mean_t spikes6  [N, 2]

Every conv is a 3x3 SAME conv on a 3x3 image, i.e. a dense linear map on the
9*C flattened features.  The whole per-timestep network is therefore a chain
of six small matmuls plus elementwise integrate-and-fire updates.

Kernel formulation (per core, pure data parallel over the batch):
  - One block-diagonal "mega" weight matrix Wblk [85 x 77] evaluates ALL six
    layers at once in a layer-pipelined (wavefront) schedule: at step k,
    layer l processes timestep t = k - (l-1).  fp32r matmuls (full-rate fp32).
  - rhs tile [85 x 1024]: rows 0..74 = spike rows (aligned with the membrane
    rows in PSUM), rows 75..83 = the 9 input pixels (static), row 84 = ones
    (bias input).  1024 samples span two PSUM banks (2 matmuls per step).
  - Membrane potentials v live in PSUM rows 0..74 and are accumulated by the
    matmul itself (start=False).  Rows 75..76 accumulate the layer-6 spikes
    scaled by 1/T (the final output) across steps - also free via matmul.
  - Default mode sigma_clamp needs only TWO elementwise passes per step, one
    per engine:
      ScalarE:  sigma = sign(v - thr) -> rhs spike rows ({-1,+1}; weights are
                rewired for s=(sigma+1)/2, so -1 rows contribute exactly 0)
      VectorE:  v = min(v, thr) - combined with a -thr*I diagonal feedback
                block in Wblk this is an EXACT hard reset: the clamp pins v
                to exactly thr at spike time, so the next step's -thr*sigma
                feedback zeroes it.
    (Caveat: at an exact fp32 tie v == thr, sign() returns 0, encoding half a
    spike; measure-zero and irrelevant at this problem's threshold margins.)
  - Warmup bias over-accumulation (each layer receives its bias on every step
    incl. the (l-1) steps before its pipeline slot becomes valid) is cancelled
    by a k=0-only weight matrix whose ones-row carries the correction.

Sharding: batch N=65536 split evenly across the 8 NeuronCores.
Measured ~145-175us per core on trn2 (vs ~450us for the naive is_ge +
copy_predicated version); exact (0.0) error vs the fp32 reference.
"""

import numpy as np

import concourse.bacc as bacc
import concourse.mybir as mybir
import concourse.tile as tile
from concourse.bass_utils import run_bass_kernel_spmd

F32 = mybir.dt.float32
F32R = mybir.dt.float32r

N_CORES = 8
N_TOTAL = 65536
N_PER_CORE = N_TOTAL // N_CORES          # 8192
TILE_N = 512                              # samples per PSUM bank (fp32 limit)
N_TILES = N_PER_CORE // TILE_N            # 16
T = 16
N_LAYERS = 6
STEPS = T + N_LAYERS - 1                  # 21 wavefront steps with valid work
# one extra matmul step so the accumulator rows pick up the last s6 spikes
MM_STEPS = STEPS + 1                      # 22

# feature rows of the membrane state (v) / spike rows
ROWS = [18, 18, 18, 9, 10, 2]             # v1..v6
ROW_OFF = np.cumsum([0] + ROWS).tolist()  # [0,18,36,54,63,73,75]
NV = ROW_OFF[-1]                          # 75
K_X = NV                                  # x rows start (75..83)
K_ONE = NV + 9                            # ones row (84)
K_TOT = NV + 9 + 1                        # 85
M_ACC = NV                                # acc cols start (75..76)
M_TOT = NV + 2                            # 77
THRESHOLDS = [20.0, 10.0, 8.0, 8.0, 30.0, 30.0]


def _conv_matrix(w):
    """3x3 SAME conv on a 3x3 image as a dense [Cout*9, Cin*9] matrix.

    Feature index = c*9 + i*3 + j; out[o] = sum_k M[o, k] * in[k].
    """
    co, ci = w.shape[0], w.shape[1]
    m = np.zeros((co * 9, ci * 9), np.float32)
    for o in range(co):
        for c in range(ci):
            for oi in range(3):
                for oj in range(3):
                    for ii in range(3):
                        for ij in range(3):
                            kh, kw = ii - oi + 1, ij - oj + 1
                            if 0 <= kh < 3 and 0 <= kw < 3:
                                m[o * 9 + oi * 3 + oj, c * 9 + ii * 3 + ij] = \
                                    w[o, c, kh, kw]
    return m


def _build_constants(w1, b1, w2, b2, w3, b3, w4, b4, wfc1, wfc2,
                     mode="basic"):
    """Wblk [K_TOT, M_TOT], thr [NV,1], vinit [NV,1] as numpy arrays.

    mode:
      basic       - spike rows carry s in {0,1} (is_ge), reset by copy_predicated
      clamp       - adds a -theta*I diagonal feedback block (spike rows -> own
                    membrane columns); with a per-step clamp v=min(v,theta)
                    this reproduces the hard reset exactly (the clamp pins v
                    to exactly theta at spike time, so subtracting theta on
                    the next step equals reset-to-zero)
      sigma_clamp - clamp feedback plus sigma encoding: spike rows carry
                    sigma = sign(v-theta) in {-1,+1} (computed on the Scalar
                    engine); since s = (sigma+1)/2, all spike-row weights are
                    halved and their row-sums/2 move into the ones-row bias.
                    With rows initialized to -1, inactive layers contribute
                    exactly zero.
    """
    mats = [
        _conv_matrix(w1),                 # 9  -> 18
        _conv_matrix(w2),                 # 18 -> 18
        _conv_matrix(w3),                 # 18 -> 18
        _conv_matrix(w4),                 # 18 -> 9
        np.asarray(wfc1, np.float32),     # 9  -> 10
        np.asarray(wfc2, np.float32),     # 10 -> 2
    ]
    biases = [
        np.repeat(np.asarray(b1, np.float32), 9),
        np.repeat(np.asarray(b2, np.float32), 9),
        np.repeat(np.asarray(b3, np.float32), 9),
        np.repeat(np.asarray(b4, np.float32), 9),
        np.zeros(10, np.float32),
        np.zeros(2, np.float32),
    ]

    wblk = np.zeros((K_TOT, M_TOT), np.float32)
    # layer 1: x rows -> v1 cols
    wblk[K_X:K_X + 9, 0:18] = mats[0].T
    # layers 2..6: spike rows of layer l-1 -> v_l cols
    for l in range(1, 6):
        r0, r1 = ROW_OFF[l - 1], ROW_OFF[l]      # spike rows (prev layer)
        c0, c1 = ROW_OFF[l], ROW_OFF[l + 1]      # v cols (this layer)
        wblk[r0:r1, c0:c1] = mats[l].T
    # s6 rows -> output accumulator cols, scaled by 1/T
    wblk[ROW_OFF[5]:ROW_OFF[6], M_ACC:M_ACC + 2] = np.eye(2, dtype=np.float32) / T
    # ones row -> biases
    for l in range(6):
        wblk[K_ONE, ROW_OFF[l]:ROW_OFF[l + 1]] = biases[l]
    if mode in ("clamp", "sigma_clamp"):
        # spike rows -> own membrane columns: subtract theta on next step
        for l in range(6):
            r0, r1 = ROW_OFF[l], ROW_OFF[l + 1]
            wblk[r0:r1, r0:r1] += -THRESHOLDS[l] * np.eye(r1 - r0,
                                                          dtype=np.float32)
    if mode == "sigma_clamp":
        # s = (sigma+1)/2: halve spike-row weights, move row-sums/2 into bias
        half = wblk[0:NV, :] * 0.5
        wblk[K_ONE, :] += half.sum(axis=0)
        wblk[0:NV, :] = half

    thr = np.zeros((NV, 1), np.float32)
    vinit = np.zeros((NV, 1), np.float32)
    for l in range(6):
        thr[ROW_OFF[l]:ROW_OFF[l + 1], 0] = THRESHOLDS[l]
        # layer l (0-indexed) gets its bias added on l warmup steps (k=0..l-1)
        # before its valid window starts at k=l; cancel them.
        vinit[ROW_OFF[l]:ROW_OFF[l + 1], 0] = -float(l) * biases[l]
    return wblk, thr, vinit


def build_program(n_tiles=N_TILES, repeat=1, elementwise=True,
                  mode="sigma_clamp", span=1024, loop_repeat=0):
    """span: samples per PSUM tile (512 = 1 bank, 1024 = 2 banks)."""
    n_samp = n_tiles * TILE_N
    assert span % TILE_N == 0 and n_samp % span == 0
    n_mm = span // TILE_N                 # matmuls per step per span-tile
    n_stiles = n_samp // span
    nc = bacc.Bacc("TRN2", target_bir_lowering=False, debug=False)

    # 10 rows: 9 pixel rows + a row of ones (bias input), pre-built on host
    xst = nc.dram_tensor("xst", [10, n_samp], F32R, kind="ExternalInput")
    wblk = nc.dram_tensor("wblk", [K_TOT, M_TOT], F32R, kind="ExternalInput")
    # k=0 weights: ones-row additionally carries the warmup-bias cancellation
    wblk0 = nc.dram_tensor("wblk0", [K_TOT, M_TOT], F32R, kind="ExternalInput")
    thr = nc.dram_tensor("thr", [NV, 1], F32, kind="ExternalInput")
    negthr = nc.dram_tensor("negthr", [NV, 1], F32, kind="ExternalInput")
    out = nc.dram_tensor("out", [2, n_samp], F32, kind="ExternalOutput")

    with tile.TileContext(nc) as tc:
        with tc.tile_pool(name="const", bufs=1) as constp, \
             tc.tile_pool(name="rhs", bufs=max(2, 8 // n_mm)) as rhsp, \
             tc.tile_pool(name="res", bufs=4) as resp, \
             tc.tile_pool(name="psum", bufs=max(2, 8 // n_mm),
                          space="PSUM") as psump:

            wblk_t = constp.tile([K_TOT, M_TOT], F32R)
            nc.sync.dma_start(wblk_t[:], wblk[:])
            wblk0_t = constp.tile([K_TOT, M_TOT], F32R)
            nc.sync.dma_start(wblk0_t[:], wblk0[:])
            thr_t = constp.tile([NV, 1], F32)
            nc.sync.dma_start(thr_t[:], thr[:])
            negthr_t = constp.tile([NV, 1], F32)
            nc.sync.dma_start(negthr_t[:], negthr[:])
            zeros_t = constp.tile([NV, TILE_N], F32)
            nc.gpsimd.memset(zeros_t[:], 0.0)

            def tile_body(j):
                rhs = rhsp.tile([K_TOT, span], F32R)
                psum = psump.tile([M_TOT, span], F32)

                # static rows: spike rows start at "no spike" (0 in s
                # encoding, -1 in sigma encoding); x pixels + ones via DMA.
                # (memset has no f32r flavor - write the bits as uint32)
                init_bits = 0xBF800000 if mode == "sigma_clamp" else 0
                nc.gpsimd.memset(rhs[0:NV, :].bitcast(mybir.dt.uint32),
                                 init_bits)
                nc.sync.dma_start(
                    rhs[K_X:K_X + 10, :],
                    xst[:, j * span:(j + 1) * span],
                )

                for k in range(MM_STEPS):
                    # The membrane state lives in PSUM across all steps: the
                    # matmul accumulates onto it (start only at k=0) while
                    # ACT/DVE read/rewrite it between steps.  That
                    # interleaving is serialized by Tile dependency tracking
                    # and is fine on HW (has_written bits persist across
                    # engine writes), but the sim's conservative group guard
                    # must be skipped.
                    w = wblk0_t if k == 0 else wblk_t
                    for m in range(n_mm):
                        nc.tensor.matmul(
                            psum[:, m * TILE_N:(m + 1) * TILE_N],
                            w[:],
                            rhs[:, m * TILE_N:(m + 1) * TILE_N],
                            start=(k == 0),
                            stop=(k == MM_STEPS - 1),
                            skip_group_check=True,
                        )
                    if k < MM_STEPS - 1 and elementwise:
                        # spikes (also feeds next matmul + acc rows)
                        if mode == "sigma_clamp":
                            # sigma = sign(v - theta), on the Scalar engine
                            nc.scalar.activation(
                                rhs[0:NV, :], psum[0:NV, :],
                                mybir.ActivationFunctionType.Sign,
                                bias=negthr_t[:], scale=1.0,
                            )
                        else:
                            nc.vector.tensor_scalar(
                                rhs[0:NV, :], psum[0:NV, :],
                                thr_t[:], None, mybir.AluOpType.is_ge,
                            )
                    if k < MM_STEPS - 2 and elementwise:
                        if mode in ("clamp", "sigma_clamp"):
                            # clamp to theta; with the -theta*I feedback in
                            # Wblk this is an exact hard reset (see above)
                            nc.vector.tensor_scalar(
                                psum[0:NV, :], psum[0:NV, :],
                                thr_t[:], None, mybir.AluOpType.min,
                            )
                        else:
                            # hard reset to zero where spiked (mask viewed as
                            # uint32: 1.0f bits nonzero, 0.0f bits zero)
                            for m in range(n_mm):
                                nc.vector.copy_predicated(
                                    psum[0:NV, m * TILE_N:(m + 1) * TILE_N],
                                    rhs[0:NV, m * TILE_N:(m + 1) * TILE_N]
                                    .bitcast(mybir.dt.uint32),
                                    zeros_t[:],
                                )

                # engines need quadrant-aligned partition bases: copy from
                # partition 64 (13 rows) and DMA out the last two rows.
                res = resp.tile([13, span], F32)
                nc.vector.tensor_copy(res[:], psum[64:M_TOT, :])
                nc.sync.dma_start(
                    out[:, j * span:(j + 1) * span],
                    res[M_ACC - 64:M_TOT - 64, :],
                )

            # timing mode (repeat > 1) statically unrolls the whole
            # computation to amortize away host/axon dispatch overhead
            if loop_repeat:
                # hardware loop around the full computation: cannot be
                # dead-code-eliminated, so wall-clock slope over
                # loop_repeat measures true device time per iteration
                with tc.For_i(0, loop_repeat, 1):
                    for j in range(n_stiles):
                        tile_body(j)
            else:
                for _ in range(repeat):
                    for j in range(n_stiles):
                        tile_body(j)

    nc.compile()
    return nc


# ---- optimized on-device firing check ------------------------------------
# Layout constants.  14 sample groups of 9 pixel rows share each matmul
# column: A-half = PE rows 0..62 (+ ones row 63), B-half = rows 64..126
# (+ ones row 127).  Out partitions 18g+f (7 groups x 18 conv1 features).
# The ones row streams a -tau weight row, so psum = conv1(x) - tau directly
# and the firing test is simply "any psum >= 0".
# Scan balance: DVE (banks 0+2) runs at 1.04ns/col with ~170ns fixed per
# instruction; ACT (bank 1) at 0.83ns/col with ~330ns fixed plus it starts
# one matmul later.  B1C=454 equalizes both engines' finish times.
CHK_B1C = 454            # B-half cols = ACT's bank (7*454 = 3178 samples)
CHK_XCOLS = (N_PER_CORE - 7 * CHK_B1C + 6) // 7        # A-half cols (717)
CHK_A2C = CHK_XCOLS - 512     # A leftover bank cols (205)
CHK_WOFF = CHK_XCOLS     # weight block starts here
CHK_NPACK = CHK_XCOLS + 126   # packed tensor [128, CHK_NPACK]
BF16 = mybir.dt.bfloat16
# check-program element type: fp8e4m3 halves the DMA bytes vs bf16; its
# rounding error is rigorously absorbed into the tau margin (REL/ABS below)
CHK_DT = mybir.dt.float8e4
CHK_REL = {mybir.dt.float8e4: 2.0 ** -4, BF16: 2.0 ** -8}[CHK_DT]
CHK_ABS = {mybir.dt.float8e4: 2.0 ** -10, BF16: 2.0 ** -100}[CHK_DT]


def build_check_program(loop_repeat=0, unroll=1, sb_bufs=4, body_unroll=2,
                        dma_scheme="alt_out"):
    """Event-driven fast path: decide on-device whether ANY sample can ever
    produce a layer-1 spike.

    An IF layer with constant per-step drive c fires iff max_t v1(t) =
    16*c >= theta1 for some feature.  conv1's drive c1 = M1 x + b1 is
    constant across time, so the whole network's activity reduces to this
    single check when the zero-input trajectory is silent (host-verified).

    v2 design, tuned against the TRN2 instruction cost model:
      - ONE input DMA: x (bf16, 14 groups of 9 pixel rows block-packed),
        the block-diagonal conv1 weights AND the -tau threshold row ride
        in a single [128, CHK_NPACK] bf16 tensor (HWDGE fixed cost ~2us
        per DMA dominates; N DMAs serialize on the HWDGE).
      - 3 matmuls (512+512+147 cols, one PSUM bank each): 7 samples per
        column, tau folded in via the ones row -> psum = c1_raw - tau.
      - The scan splits across two engines reading DIFFERENT psum banks in
        parallel: DVE reduce_max on banks 0+2, ACT sign+accumulate on bank
        1 (accum == -512 exactly iff every column is silent).
      - ONE tiny output DMA [126, 3] with both engines' verdicts.
      - No gpsimd/Pool usage, no zero-output DMA (the host emits the zeros).
    """
    nc = bacc.Bacc("TRN2", target_bir_lowering=False, debug=False)
    xpk = nc.dram_tensor("xpk", [128, CHK_NPACK], CHK_DT, kind="ExternalInput")
    flagout = nc.dram_tensor("flagout", [126, 3], F32, kind="ExternalOutput")

    with tile.TileContext(nc) as tc:
        with tc.tile_pool(name="const", bufs=1) as constp, \
             tc.tile_pool(name="sb", bufs=sb_bufs) as sb, \
             tc.tile_pool(name="ps", bufs=2, space="PSUM") as ps:
            # hoist the ACT Sign table load off the critical path: a dummy
            # 1-elem activation at t=0 (under the input DMA) loads the
            # function set once; later Sign activations reuse it.
            scratch = constp.tile([1, 2], F32)
            nc.gpsimd.memset(scratch[:], 0.0)
            nc.scalar.activation(scratch[0:1, 1:2], scratch[0:1, 0:1],
                                 mybir.ActivationFunctionType.Sign,
                                 bias=0.0, scale=1.0)

            def check_body(_i):
                xt = sb.tile([128, CHK_NPACK], CHK_DT, name="xt", tag="xt")
                in_eng = (nc.scalar if (dma_scheme == "alt_both" and _i % 2)
                          else nc.sync)
                in_eng.dma_start(xt[:], xpk[:])
                # one PSUM tile per bank so each scan depends only on its
                # own matmul (a shared tile serializes reads behind every
                # write to it)
                ps0 = ps.tile([126, 512], F32, name="ps0", tag="ps0")
                ps1 = ps.tile([126, CHK_B1C], F32, name="ps1", tag="ps1")
                ps2 = ps.tile([126, CHK_A2C], F32, name="ps2", tag="ps2")
                wA = xt[0:64, CHK_WOFF:CHK_WOFF + 126]
                wB = xt[64:128, CHK_WOFF:CHK_WOFF + 126]
                nc.tensor.matmul(ps0[:], wA, xt[0:64, 0:512],
                                 start=True, stop=True, tile_position=(0, 0))
                nc.tensor.matmul(ps1[:], wB, xt[64:128, 0:CHK_B1C],
                                 start=True, stop=True, tile_position=(64, 0))
                nc.tensor.matmul(ps2[:], wA, xt[0:64, 512:CHK_XCOLS],
                                 start=True, stop=True, tile_position=(0, 0))
                flags = sb.tile([126, 3], F32, name="flags", tag="flags")
                nc.vector.reduce_max(flags[:, 0:1], ps0[:],
                                     axis=mybir.AxisListType.X)
                # ACT: sigma = sign(psum) in place, accum = sum(sigma);
                # silent bank <=> accum == -512 exactly
                nc.scalar.activation(ps1[:], ps1[:],
                                     mybir.ActivationFunctionType.Sign,
                                     bias=0.0, scale=1.0,
                                     accum_out=flags[:, 2:3])
                nc.vector.reduce_max(flags[:, 1:2], ps2[:],
                                     axis=mybir.AxisListType.X)
                # flag DMA alternates between the SP and ACT HWDGE rings so
                # SP's stream is back-to-back input DMAs: with both DMAs on
                # SP, iteration k+1's input dispatch queues behind iteration
                # k's output (which waits on k's scans), fully serializing
                # the loop (~2.4us/iter -> ~1.3us/iter)
                if dma_scheme == "alt_both":     # in/out on opposite rings
                    eng = nc.sync if _i % 2 else nc.scalar
                elif dma_scheme == "alt_out":    # out alternates rings
                    eng = nc.scalar if _i % 2 else nc.sync
                elif dma_scheme == "act_out":    # out always on ACT ring
                    eng = nc.scalar
                else:                            # sync_out
                    eng = nc.sync
                eng.dma_start(flagout[:], flags[:])

            if loop_repeat:
                assert loop_repeat % body_unroll == 0, \
                    "loop_repeat must divide body_unroll"
                with tc.For_i(0, loop_repeat // body_unroll, 1):
                    for i in range(body_unroll):
                        check_body(i)
            elif unroll > 1:
                for i in range(unroll):
                    check_body(i)
            else:
                check_body(0)

    nc.compile()
    return nc


def build_check_program_raw():
    """Raw-bass (no TileContext) variant of the check program: hand-rolled
    semaphores instead of Tile's start/drain barrier cascade (~1.1us of the
    single-shot critical path in the TRN2 cost model).

    Protocol (all sems 0 at entry, restored to 0 at exit so repeated NEFF
    executions stay correct):
      SP:   dma xpk->xt (+16 s_dma) ; wait s_scan>=3 ; dma flags->flagout
            (+16 s_out) ; wait s_out>=16 ; clear all sems
      Pool: memset scratch (+1 s_z)
      ACT:  wait s_z ; dummy Sign (hoists the 1.3us act-table load under
            the input DMA) ; wait s_mm>=2 ; Sign+accum on bank1 (+1 s_scan)
      PE:   wait s_dma>=16 ; matmul ps0/ps1/ps2 (+1 s_mm each)
      DVE:  wait s_mm>=1 ; rmax ps0 (+1 s_scan) ; wait s_mm>=3 ; rmax ps2
            (+1 s_scan)
    """
    import concourse.bass as bass
    nc = bacc.Bacc("TRN2", target_bir_lowering=False, debug=False)
    xpk = nc.dram_tensor("xpk", [128, CHK_NPACK], CHK_DT, kind="ExternalInput")
    flagout = nc.dram_tensor("flagout", [126, 3], F32, kind="ExternalOutput")

    with nc.semaphore("s_dma") as s_dma, \
         nc.semaphore("s_mm") as s_mm, \
         nc.semaphore("s_scan") as s_scan, \
         nc.semaphore("s_out") as s_out, \
         nc.sbuf_tensor("xt", [128, CHK_NPACK], CHK_DT) as xt, \
         nc.sbuf_tensor("flags", [126, 3], F32) as flags, \
         nc.sbuf_tensor("scratch", [1, 2], F32) as scratch, \
         nc.psum_tensor("ps0", [126, 512], F32) as ps0, \
         nc.psum_tensor("ps1", [126, CHK_B1C], F32) as ps1, \
         nc.psum_tensor("ps2", [126, CHK_A2C], F32) as ps2:

        # manual BassBlock with a barrier-free exit: for a single-block
        # program the exit all-engine barrier (~280ns after the final DMA
        # wait) only serves block composition; each engine halting at the
        # end of its own drained stream is sufficient
        block = bass.BassBlock(nc, f"chk_{nc.next_id()}", no_gpsimd_drain=True)
        nc.cur_block = block
        if True:

            @block.sync
            def _(sync):
                # bass's preamble re-clears all kernel sems at the start of
                # every NEFF execution, so no explicit restore is needed
                sync.dma_start(xt[:], xpk[:]).then_inc(s_dma, 16)
                sync.wait_ge(s_scan, 4)
                # stall SP until the flag DMA's completion sem fires: ~300ns
                # of tail in the cost model, but guarantees flagout is in
                # DRAM before the NEFF reports done.  (Without it, a runtime
                # that reads outputs immediately at engine-halt could see a
                # stale all-zero flagout -> false fire -> the ~150us dense
                # fallback.  Tile programs always wait; match that.)
                sync.dma_start(flagout[:], flags[:]).then_inc(s_out, 16)
                sync.wait_ge(s_out, 16)

            @block.scalar
            def _(scalar):
                scalar.wait_ge(s_scan, 1)
                scalar.activation(scratch[0:1, 1:2], scratch[0:1, 0:1],
                                  mybir.ActivationFunctionType.Sign,
                                  bias=0.0, scale=1.0)
                scalar.wait_ge(s_mm, 2)
                scalar.activation(ps1[:], ps1[:],
                                  mybir.ActivationFunctionType.Sign,
                                  bias=0.0, scale=1.0,
                                  accum_out=flags[:, 2:3]).then_inc(s_scan, 1)

            @block.tensor
            def _(tensor):
                tensor.wait_ge(s_dma, 16)
                tensor.matmul(ps0[:], xt[0:64, CHK_WOFF:CHK_WOFF + 126],
                              xt[0:64, 0:512], start=True, stop=True,
                              tile_position=(0, 0)).then_inc(s_mm, 1)
                tensor.matmul(ps1[:], xt[64:128, CHK_WOFF:CHK_WOFF + 126],
                              xt[64:128, 0:CHK_B1C], start=True, stop=True,
                              tile_position=(64, 0)).then_inc(s_mm, 1)
                tensor.matmul(ps2[:], xt[0:64, CHK_WOFF:CHK_WOFF + 126],
                              xt[0:64, 512:CHK_XCOLS], start=True, stop=True,
                              tile_position=(0, 0)).then_inc(s_mm, 1)

            @block.vector
            def _(vector):
                # DVE's first instruction: init the ACT warmup scratch (its
                # s_scan inc is guaranteed first since DVE runs in order)
                vector.memset(scratch[:], 0.0).then_inc(s_scan, 1)
                vector.wait_ge(s_mm, 1)
                vector.reduce_max(flags[:, 0:1], ps0[:],
                                  axis=mybir.AxisListType.X).then_inc(s_scan, 1)
                vector.wait_ge(s_mm, 3)
                vector.reduce_max(flags[:, 1:2], ps2[:],
                                  axis=mybir.AxisListType.X).then_inc(s_scan, 1)

        # barrier-free Block exit (mirrors BassBlock.__exit__ minus
        # all_engine_barrier): branch each engine to the end block and
        # drain the non-GpSimd engines
        for engine, last_body in block.last_body.items():
            with nc.body(last_body, parent=nc.cur_bb,
                         allow_existing_parent=True):
                engine.br(block.end_bb)
        nc.switch_bb(block.end_bb)
        for eng_type, eng in nc.engines.items():
            if eng_type == mybir.EngineType.Pool:
                continue
            d = mybir.InstDrain(name=nc.get_next_instruction_name(),
                                ins=[], outs=[], bass_is_fusable=False)
            d.engine = eng_type
            eng.add_instruction(d)
        nc.cur_block = None

    nc.compile()
    return nc


def _check_tau_and_err(w1, b1, maxabs_x):
    """Per-feature device threshold tau and its soundness margin.

    Device flags iff conv1_raw(x)_f >= tau_f for some sample/feature, where
    tau_f = (theta1 - EPS)/16 - b1_f.  EPS rigorously covers the CHK_DT
    rounding of x and M1 (relative CHK_REL, subnormal floor CHK_ABS; the
    M1 term is computed exactly from the actual cast), fp32 accumulation-
    order drift vs the jax fp32 reference conv, and the reference's T
    sequential adds.  tau itself is pre-shifted one rounding bound DOWN so
    its own CHK_DT cast stays conservative.
    """
    npdt = mybir.dt.np(CHK_DT)
    m1 = _conv_matrix(np.asarray(w1, np.float32))          # [18, 9]
    m1c = m1.astype(npdt).astype(np.float32)               # device weights
    b1r = np.repeat(np.asarray(b1, np.float32), 9)         # [18]
    err = (np.abs(m1c).sum(axis=1) * (float(maxabs_x) * CHK_REL + CHK_ABS)
           + np.abs(m1c - m1).sum(axis=1) * float(maxabs_x)
           + 1e-3)                                         # [18]
    tau = (THRESHOLDS[0] / 16.0) - b1r - err               # [18]
    tau_down = tau - (np.abs(tau) * CHK_REL + CHK_ABS) - 1e-6
    return tau_down.astype(np.float32)


def make_check_in_maps(x, w1, b1):
    """Per-core packed [128, CHK_NPACK] CHK_DT inputs for the check program."""
    npdt = mybir.dt.np(CHK_DT)
    xs = np.asarray(x, np.float32).reshape(N_TOTAL, 9)
    maxabs_x = float(np.abs(xs).max())
    tau = _check_tau_and_err(w1, b1, maxabs_x)             # [18]
    m1 = _conv_matrix(np.asarray(w1, np.float32))          # [18, 9]

    # weight block [128, 126]: rows 9g+k -> psum partition 18g+f carries
    # M1[f, k]; ones rows 63/127 carry -tau (repeated per group)
    wblk = np.zeros((128, 126), np.float32)
    for g in range(7):
        wblk[9 * g:9 * g + 9, 18 * g:18 * g + 18] = m1.T
        wblk[64 + 9 * g:64 + 9 * g + 9, 18 * g:18 * g + 18] = m1.T
    wblk[63] = np.tile(-tau, 7)
    wblk[127] = np.tile(-tau, 7)

    in_maps = []
    for core in range(N_CORES):
        shard = xs[core * N_PER_CORE:(core + 1) * N_PER_CORE]   # [8192, 9]
        # A-half: samples 0..4612 as [7, 659, 9]; B-half: samples
        # 4613..8191 (+5 dups) as [7, 512, 9]; B cols 512.. are never read
        # by a matmul - fill with sample 0.
        na = 7 * CHK_XCOLS
        a = shard[np.minimum(np.arange(na), N_PER_CORE - 1)]
        a = a.reshape(7, CHK_XCOLS, 9)
        bidx = np.minimum(na + np.arange(7 * CHK_B1C), N_PER_CORE - 1)
        b = shard[bidx].reshape(7, CHK_B1C, 9)
        pk = np.zeros((128, CHK_NPACK), np.float32)
        pk[0:63, 0:CHK_XCOLS] = a.transpose(0, 2, 1).reshape(63, CHK_XCOLS)
        pk[64:127, 0:CHK_B1C] = b.transpose(0, 2, 1).reshape(63, CHK_B1C)
        pk[64:127, CHK_B1C:CHK_XCOLS] = np.tile(shard[0], 7).reshape(63, 1)
        pk[63, 0:CHK_XCOLS] = 1.0
        pk[127, 0:CHK_XCOLS] = 1.0
        pk[:, CHK_WOFF:] = wblk
        in_maps.append({"xpk": pk.astype(npdt)})
    return in_maps


def check_flags_fire(results):
    """Host-side verdict from the check program's flagout tensors."""
    for r in results:
        f = np.asarray(r["flagout"], np.float32)           # [126, 3]
        if (f[:, 0] >= 0.0).any() or (f[:, 1] >= 0.0).any():
            return True
        if (f[:, 2] > -(CHK_B1C - 0.5)).any():
            return True
    return False


def quiet_zero_input(b1, b2, b3, b4):
    """Host check: with zero input spikes, is every layer silent (with
    margin)?  If layers 1..l-1 are silent a sample's layer-l potential is
    t*b_l, so silence of the bias-only trajectory is checked layer by
    layer.  Margin 1e-2 absorbs any fp32 accumulation drift."""
    for b, thr in zip((np.asarray(b1), np.asarray(b2), np.asarray(b3),
                       np.asarray(b4)), THRESHOLDS[:4]):
        drive = float(np.maximum(np.asarray(b, np.float32), 0.0).max())
        if T * drive >= thr - 1e-2:
            return False
    return True   # fc layers have zero bias in this architecture



_PROGRAM_CACHE = {}


def _get_program():
    if "nc" not in _PROGRAM_CACHE:
        _PROGRAM_CACHE["nc"] = build_program()
    return _PROGRAM_CACHE["nc"]


def _get_check_program():
    if "chk" not in _PROGRAM_CACHE:
        # raw-bass variant: no Tile barrier cascade on the single-shot
        # critical path (the Tile build_check_program remains for the
        # loop_repeat timing mode used by test.py)
        _PROGRAM_CACHE["chk"] = build_check_program_raw()
    return _PROGRAM_CACHE["chk"]


def make_in_maps(x, w1, b1, w2, b2, w3, b3, w4, b4, wfc1, wfc2,
                 mode="sigma_clamp"):
    wblk, thr, vinit = _build_constants(
        np.asarray(w1, np.float32), np.asarray(b1, np.float32),
        np.asarray(w2, np.float32), np.asarray(b2, np.float32),
        np.asarray(w3, np.float32), np.asarray(b3, np.float32),
        np.asarray(w4, np.float32), np.asarray(b4, np.float32),
        np.asarray(wfc1, np.float32), np.asarray(wfc2, np.float32),
        mode=mode)
    wblk0 = wblk.copy()
    wblk0[K_ONE, 0:NV] += vinit[:, 0]
    xs = np.asarray(x, np.float32).reshape(N_TOTAL, 9)
    in_maps = []
    for c in range(N_CORES):
        shard = xs[c * N_PER_CORE:(c + 1) * N_PER_CORE]
        xst = np.ones((10, N_PER_CORE), np.float32)
        xst[0:9] = shard.T
        in_maps.append({
            "xst": xst,
            "wblk": wblk,
            "wblk0": wblk0,
            "thr": thr,
            "negthr": -thr,
        })
    return in_maps


def kernel(x, w1, b1, w2, b2, w3, b3, w4, b4, wfc1, wfc2, T=16, **_):
    assert int(T) == 16, "kernel is specialized for T=16"
    # Event-driven fast path: when the bias-only trajectory is silent
    # (weights-derived, host-checked), network activity reduces to the
    # per-sample layer-1 firing condition 16*c1 >= theta1, checked on
    # device.  If no sample can fire, the output is exactly zero.
    if quiet_zero_input(b1, b2, b3, b4):
        chk = _get_check_program()
        chk_maps = make_check_in_maps(x, w1, b1)
        res = run_bass_kernel_spmd(chk, chk_maps,
                                   core_ids=list(range(N_CORES)))
        # device computed psum = conv1_raw(x) - tau with tau folded into
        # the matmul; fire iff any psum >= 0 (DVE rmax banks 0+2) or the
        # ACT sign-accumulator of bank 1 departs from exactly -512
        if not check_flags_fire(res.results):
            return np.zeros((N_TOTAL, 2), np.float32)
    # exact dense path (any potentially-firing sample, or noisy biases)
    nc = _get_program()
    in_maps = make_in_maps(x, w1, b1, w2, b2, w3, b3, w4, b4, wfc1, wfc2,
                           mode="sigma_clamp")
    res = run_bass_kernel_spmd(nc, in_maps, core_ids=list(range(N_CORES)))
    out = np.empty((N_TOTAL, 2), np.float32)
    for c in range(N_CORES):
        out[c * N_PER_CORE:(c + 1) * N_PER_CORE] = res.results[c]["out"].T
    return out

